# revision 10
# baseline (speedup 1.0000x reference)
"""Trainium2 Bass kernel: Performer (linear) attention + in/out projections.

Problem nn_LinearPerformerAttention_6717328851263:
  x:(4,4096,1024) f32, w_qkv:(1024,3072), proj_matrix:(16,64,256),
  w_out:(1024,1024), b_out:(1024,)

  qkv = x @ w_qkv ; split q,k,v ; per (b,h): q_proj=elu1(q@P_h), k_proj=elu1(k@P_h)
  kv = k_proj^T v ; k_sum = sum_n k_proj ; attn = (q_proj @ kv) / (q_proj@k_sum)
  out = attn @ w_out + b_out

Sharding over 8 cores: core c -> (batch b=c//2, head-group g=c%2: 8 of 16 heads).
Each core computes partial y_c = attn(b, heads_g) @ w_out[512g:512g+512, :].
Host gather: out[b] = y_(b,0) + y_(b,1) + b_out.

v1 rewrite vs baseline (594 us):
- all matmul operands bf16 (rel_fro ~3.5e-3 vs 2e-2 gate, CPU-simulated):
  f32r pays 4x cycles on <256-wide streams and the TRN2 PE p-state ramp
  (0.65/1.2/2.4 GHz; max only after 3us of CONTINUOUS execution) punishes
  any stall; bf16 is 1 cycle/row at every width.
- qT kept SBUF-resident for pass B (kills the 16 MiB DRAM spill round trip).
- kv state computed directly in [F,d] orientation (lhsT=k_projE, rhs=v|1):
  65-row streams, ~2x fewer PE cycles than the old [d,F]+transpose fixup,
  and the fixup transposes disappear (ksr is built with one tensor_scalar).
- 3-stage software pipeline in both passes: PE stream for iteration i is
  [independent GEMMs for group g | proj mms for g-1 | consumer mms for g-2]
  so matmuls never wait on the elu chain; PSUM tiles are drained to SBUF
  bf16 right after production (PSUM can only hold ~8 [128,512] tiles).
- elu1(x)=min(exp(x),1)+relu(x): exp on Scalar, relu split Scalar/Vector,
  min+add on Vector.  All elu intermediates stay f32 (DVE ops with 2-byte
  INPUTS hit a ~10x slow path; f32-in/bf16-out runs at full rate), bf16 is
  written only at the matmul-input boundary.  GpSimd runs nothing bulky
  (Q7 tensor routines are ~12x slower than DVE).
"""

import numpy as np
from contextlib import ExitStack

import ml_dtypes

import concourse.bass as bass
import concourse.bacc as bacc
import concourse.tile as tile
from concourse import mybir
from concourse.bass_utils import run_bass_kernel_spmd

FP32 = mybir.dt.float32
BF16 = mybir.dt.float16  # fp16: DVE-native 16-bit (bf16 inputs hit a slow DVE path)
AL = mybir.AluOpType
AF = mybir.ActivationFunctionType

B, SEQ, D = 4, 4096, 1024
H, HD, F = 16, 64, 256
HPC = 8            # heads per core
DH = HPC * HD      # 512 head-space dims per core
P = 128
NCORES = 8
GS = 512           # tokens per group
TPG = 4            # 128-token tiles per group


def _emit(tc, n, xT, wq, wk, wv, proj, wout, y):
    nc = tc.nc
    NG = n // GS

    def copy_op(idx):
        # alternate PSUM->SBUF eviction between Scalar (activation Copy)
        # and Vector engines
        return nc.scalar.copy if idx % 2 == 0 else nc.vector.tensor_copy

    ctx = ExitStack()
    with ctx:
        const = ctx.enter_context(tc.tile_pool(name="const", bufs=1))

        ones_bf = const.tile([P, P], BF16, tag="ones", name="ones")
        nc.vector.memset(ones_bf, 1.0)
        ones_f32 = const.tile([P, HD], FP32, tag="onesf", name="onesf")
        nc.vector.memset(ones_f32, 1.0)

        # proj, pair-packed [128, 256]: head 2i at partitions 0:64, head
        # 2i+1 at 64:128 (lhsT/rhs partition bases always match).
        proj_pair = [const.tile([P, F], BF16, tag=f"projp{i}", name=f"projp{i}")
                     for i in range(4)]
        for i in range(4):
            nc.sync.dma_start(out=proj_pair[i], in_=proj[i * P:(i + 1) * P, :])

        # attn lhsT, zero-padded so a head pair accumulates into one
        # [128,512] PSUM tile: kvS[h][s] [128 F-slab, 128]; cols (h%2)*64..
        # hold kv_h, other 64 cols zero.  ksr[h][s]: same but columns
        # replicate k_sum_h (denominator lands on matching partitions).
        kvS = [[const.tile([P, P], BF16, tag=f"kvS{h}_{s}", name=f"kvS{h}_{s}")
                for s in range(2)] for h in range(HPC)]
        ksr = [[const.tile([P, P], BF16, tag=f"ksr{h}_{s}", name=f"ksr{h}_{s}")
                for s in range(2)] for h in range(HPC)]
        for h in range(HPC):
            for s in range(2):
                nc.gpsimd.memset(kvS[h][s], 0.0)
                nc.gpsimd.memset(ksr[h][s], 0.0)

        # kv accumulator per head: [128 F-sub, 2 s-slabs, 65] f32.
        # col 64 = k_sum (ones column of vone).
        kv_acc = [const.tile([P, 2, HD + 1], FP32, tag=f"kva{h}", name=f"kva{h}")
                  for h in range(HPC)]

        # full-sequence qT, pair-packed [128, 4 pairs, n] fp16 (4 MiB)
        qT_sb = const.tile([P, 4, n], BF16, tag="qTs", name="qTs")

        # pass-B qproj elu pools live at ctx level: the last two pass-A
        # iterations (PE nearly idle) already emit qproj for groups 0/1
        qelupool = ctx.enter_context(tc.tile_pool(name="qelupool", bufs=4))
        qppool = ctx.enter_context(tc.tile_pool(name="qppool", bufs=3))
        qPs = {}   # (g, h, s) -> qP tile fp16

        def unit_qproj(g, h, s, idx, psum_pool, ptag="qp"):
            g0 = g * GS
            hp, hb = h // 2, (h % 2) * HD
            qp = psum_pool.tile([P, GS], FP32, tag=ptag, name=ptag)
            nc.tensor.matmul(
                qp, lhsT=(proj_pair[hp][hb:hb + HD, s * P:(s + 1) * P]),
                rhs=(qT_sb[hb:hb + HD, hp, g0:g0 + GS]),
                start=True, stop=True)
            qE = qelupool.tile([P, GS], BF16, tag="qE", name="qE")
            qR = qelupool.tile([P, GS], BF16, tag="qR", name="qR")
            nc.scalar.activation(qE, qp, AF.Exp)
            nc.scalar.activation(qR, qp, AF.Relu)
            qP = qppool.tile([P, GS], BF16, tag=f"qP{h}_{s}",
                             name=f"qP{h}_{s}")
            nc.vector.scalar_tensor_tensor(
                qP, in0=qE, scalar=1.0, in1=qR, op0=AL.min, op1=AL.add)
            qPs[(g, h, s)] = qP

        # ---------------- pass A ----------------
        # 3-stage software pipeline; within an iteration the PE stream
        # interleaves group g's qkv GEMM units with group g-1's kproj units
        # (so elu1 chases a slow trickle of PSUM tiles instead of a burst)
        # and ends with group g-2's kv matmuls (whose kP inputs got a full
        # iteration of elu latency).
        with ExitStack() as actx:
            wpool = actx.enter_context(tc.tile_pool(name="wpool", bufs=1))
            wq_sb = [wpool.tile([P, DH], BF16, tag=f"wq{s}", name=f"wq{s}") for s in range(8)]
            wk_sb = [wpool.tile([P, DH], BF16, tag=f"wk{s}", name=f"wk{s}") for s in range(8)]
            wv_sb = [wpool.tile([P, DH], BF16, tag=f"wv{s}", name=f"wv{s}") for s in range(8)]
            # spread weight loads over three DMA queues; unit_xt(0) issues
            # the first x tile on the sync queue before these run
            for s in range(8):
                nc.scalar.dma_start(out=wq_sb[s], in_=wq[s * P:(s + 1) * P, :])
                nc.gpsimd.dma_start(out=wk_sb[s], in_=wk[s * P:(s + 1) * P, :])
                (nc.scalar if s % 2 else nc.gpsimd).dma_start(
                    out=wv_sb[s], in_=wv[s * P:(s + 1) * P, :])

            xtpool = actx.enter_context(tc.tile_pool(name="xtpool", bufs=2))
            ktpool = actx.enter_context(tc.tile_pool(name="ktpool", bufs=2))
            vpool = actx.enter_context(tc.tile_pool(name="vpool", bufs=3))
            elupool = actx.enter_context(tc.tile_pool(name="elupool", bufs=4))
            kppool = actx.enter_context(tc.tile_pool(name="kppool", bufs=2))
            mmps = actx.enter_context(tc.tile_pool(name="mmps", bufs=2, space="PSUM"))
            cpps = actx.enter_context(tc.tile_pool(name="cpps", bufs=4, space="PSUM"))
            kvps = actx.enter_context(tc.tile_pool(name="kvps", bufs=2, space="PSUM"))

            xT_v = xT.rearrange("(s p) m -> p s m", p=P)

            kts = {}    # g -> kt tile [128, 4, 512]
            vones = {}  # g -> vone tile [128, 4, 8, 65]
            kPs = {}    # (g, hp, tp, h) -> kP tile [128, 512] bf16

            def unit_xt(g):
                g0 = g * GS
                xt = xtpool.tile([P, 8, GS], BF16, tag="xt", name="xt")
                nc.sync.dma_start(out=xt, in_=xT_v[:, :, g0:g0 + GS])
                kts[g] = ktpool.tile([P, 4, GS], BF16, tag="kt", name="kt")
                vone = vpool.tile([P, TPG, HPC, HD + 1], BF16, tag="vone",
                                  name="vone")
                nc.vector.tensor_copy(
                    vone[:, :, :, HD],
                    ones_bf[:, 0:TPG * HPC].rearrange("p (t h) -> p t h", t=TPG))
                vones[g] = vone
                return xt

            def unit_q(g, xt, fs):
                g0 = g * GS
                ps = mmps.tile([P, GS], FP32, tag="mm", name="mm")
                for s in range(8):
                    nc.tensor.matmul(
                        ps, lhsT=(wq_sb[s][:, fs * P:(fs + 1) * P]),
                        rhs=(xt[:, s, :]), start=(s == 0), stop=(s == 7))
                copy_op(fs)(qT_sb[:, fs, g0:g0 + GS], ps)

            def unit_k(g, xt, fs):
                ps = mmps.tile([P, GS], FP32, tag="mm", name="mm")
                for s in range(8):
                    nc.tensor.matmul(
                        ps, lhsT=(wk_sb[s][:, fs * P:(fs + 1) * P]),
                        rhs=(xt[:, s, :]), start=(s == 0), stop=(s == 7))
                copy_op(fs + 1)(kts[g][:, fs, :], ps)

            def unit_v(g, xt, t):
                ps = mmps.tile([P, GS], FP32, tag="mm", name="mm")
                for s in range(8):
                    nc.tensor.matmul(
                        ps, lhsT=(xt[:, s, t * P:(t + 1) * P]),
                        rhs=(wv_sb[s]), start=(s == 0), stop=(s == 7))
                copy_op(t)(
                    vones[g][:, t, :, 0:HD],
                    ps.rearrange("p (h e) -> p h e", h=HPC))

            def unit_kproj(g, hp, tp, h, idx):
                # c[tokens, 2ti x 256F] = k_h @ P_h, then elu1 -> kP bf16.
                # Even/odd heads sit at base partitions 0/64 so the PE runs
                # them in disjoint row groups.
                hb = (h % 2) * HD
                kt = kts[g]
                c = cpps.tile([P, GS], FP32, tag="kc", name="kc")
                for ti in range(2):
                    t = tp * 2 + ti
                    nc.tensor.matmul(
                        c[:, ti * F:(ti + 1) * F],
                        lhsT=(kt[hb:hb + HD, hp, t * P:(t + 1) * P]),
                        rhs=(proj_pair[hp][hb:hb + HD, :]),
                        start=True, stop=True)
                kE = elupool.tile([P, GS], BF16, tag="kE", name="kE")
                kR = elupool.tile([P, GS], BF16, tag="kR", name="kR")
                nc.scalar.activation(kE, c, AF.Exp)
                if idx % 16 < 10:  # balance Scalar vs Vector load
                    nc.scalar.activation(kR, c, AF.Relu)
                else:
                    nc.vector.tensor_scalar_max(kR, c, 0.0)
                kP = kppool.tile([P, GS], BF16, tag=f"kP{hp}_{tp}_{h % 2}",
                                 name=f"kP{hp}_{tp}_{h % 2}")
                nc.vector.scalar_tensor_tensor(
                    kP, in0=kE, scalar=1.0, in1=kR, op0=AL.min, op1=AL.add)
                kPs[(g, hp, tp, h)] = kP

            def unit_kv(g, hp, h):
                # kv[f, d] += k_projE^T [v|1] per (head, F-slab), PSUM
                # accumulated over the 4 token tiles, folded into kv_acc f32.
                vone = vones[g]
                kv_ps = kvps.tile([P, 2, P], FP32, tag="kv", name="kv")
                for s in range(2):
                    for t in range(TPG):
                        tp, ti = t // 2, t % 2
                        kP = kPs.pop((g, hp, tp, h)) if s == 1 and t == TPG - 1 \
                            else kPs[(g, hp, tp, h)]
                        nc.tensor.matmul(
                            kv_ps[:, s, 0:HD + 1],
                            lhsT=(kP[:, ti * F + s * P: ti * F + s * P + P]),
                            rhs=(vone[:, t, h, :]),
                            start=(t == 0), stop=(t == TPG - 1),
                            skip_group_check=True)
                if g == 0:
                    nc.vector.tensor_copy(kv_acc[h], kv_ps[:, :, 0:HD + 1])
                else:
                    nc.vector.tensor_tensor(
                        out=kv_acc[h], in0=kv_ps[:, :, 0:HD + 1],
                        in1=kv_acc[h], op=AL.add)

            for it in range(NG + 2):
                g = it            # group doing qkv GEMMs
                gk = it - 1       # group doing kproj+elu
                gv = it - 2       # group doing kv accumulation
                xt = unit_xt(g) if g < NG else None
                # 12 qkv units interleaved with 16 kproj units
                kp_units = []
                if 0 <= gk < NG:
                    kp_units = [(hp, tp, h)
                                for hp in range(HPC // 2)
                                for tp in range(2)
                                for h in (2 * hp, 2 * hp + 1)]
                qkv_units = []
                if g < NG:
                    qkv_units = [("q", fs) for fs in range(4)] + \
                                [("k", fs) for fs in range(4)] + \
                                [("v", t) for t in range(4)]
                ik = 0
                for iu, u in enumerate(qkv_units):
                    kind, a = u
                    if kind == "q":
                        unit_q(g, xt, a)
                    elif kind == "k":
                        unit_k(g, xt, a)
                    else:
                        unit_v(g, xt, a)
                    # ~16 kproj units spread over 12 qkv units
                    while len(kp_units) * 12 > (11 - iu) * 16 and kp_units:
                        hp, tp, h = kp_units.pop(0)
                        unit_kproj(gk, hp, tp, h, ik)
                        ik += 1
                for hp, tp, h in kp_units:
                    unit_kproj(gk, hp, tp, h, ik)
                    ik += 1
                if 0 <= gv < NG:
                    for hp in range(HPC // 2):
                        for h in (2 * hp, 2 * hp + 1):
                            unit_kv(gv, hp, h)
                # tail iterations: PE is nearly idle, pre-run pass-B qproj
                # for groups 0/1 out of the mmps psum pool
                if it >= NG:
                    gq = it - NG
                    for iq, (h, s) in enumerate(
                            [(h, s) for h in range(HPC) for s in range(2)]):
                        unit_qproj(gq, h, s, iq, mmps, ptag="mm")

        # ---------------- kv fixup: kv_acc -> kvS/ksr (bf16) ----------------
        for h in range(HPC):
            hb = (h % 2) * HD
            for s in range(2):
                nc.vector.tensor_copy(kvS[h][s][:, hb:hb + HD],
                                      kv_acc[h][:, s, 0:HD])
                # ksr cols hb:hb+64 = k_sum[f] broadcast along free dim
                # (scalar engine: copy of ones scaled per-partition by k_sum)
                nc.scalar.activation(
                    ksr[h][s][:, hb:hb + HD], ones_f32, AF.Copy,
                    scale=kv_acc[h][:, s, HD:HD + 1])

        # ---------------- pass B ----------------
        # same 3-stage pipeline; qproj units trickle between attn/y units.
        with ExitStack() as bctx:
            wopool = bctx.enter_context(tc.tile_pool(name="wopool", bufs=1))
            wo_sb = [wopool.tile([P, D], BF16, tag=f"wo{s}", name=f"wo{s}") for s in range(4)]
            for s in range(4):
                nc.sync.dma_start(out=wo_sb[s], in_=wout[s * P:(s + 1) * P, :])

            attpool = bctx.enter_context(tc.tile_pool(name="attpool", bufs=3))
            zpool = bctx.enter_context(tc.tile_pool(name="zpool", bufs=2))
            ypool = bctx.enter_context(tc.tile_pool(name="ypool", bufs=2))
            qpps = bctx.enter_context(tc.tile_pool(name="qpps", bufs=2, space="PSUM"))
            atps = bctx.enter_context(tc.tile_pool(name="atps", bufs=2, space="PSUM"))
            dnps = bctx.enter_context(tc.tile_pool(name="dnps", bufs=2, space="PSUM"))
            yps = bctx.enter_context(tc.tile_pool(name="yps", bufs=2, space="PSUM"))

            atts = {}  # (g, hp) -> att_sb tile fp16

            def unit_attn(g, hp):
                aps = atps.tile([P, GS], FP32, tag="at", name="at")
                dps = dnps.tile([P, GS], FP32, tag="dn", name="dn")
                for h in (2 * hp, 2 * hp + 1):
                    first = h % 2 == 0
                    last = h % 2 == 1
                    for s in range(2):
                        qP = qPs.pop((g, h, s)) if last and s == 1 \
                            else qPs[(g, h, s)]
                        nc.tensor.matmul(
                            aps, lhsT=(kvS[h][s]), rhs=(qP),
                            start=(first and s == 0), stop=(last and s == 1),
                            skip_group_check=True)
                        nc.tensor.matmul(
                            dps, lhsT=(ksr[h][s]), rhs=(qP),
                            start=(first and s == 0), stop=(last and s == 1),
                            skip_group_check=True)
                zb = zpool.tile([P, GS], FP32, tag="zb", name="zb")
                nc.vector.reciprocal_approx_fast(zb, dps)
                att = attpool.tile([P, GS], BF16, tag=f"att{hp}",
                                   name=f"att{hp}")
                nc.vector.tensor_tensor(out=att, in0=aps, in1=zb, op=AL.mult)
                atts[(g, hp)] = att

            def unit_y(g, att, t):
                g0 = g * GS
                pso = [yps.tile([P, GS], FP32, tag="yp", name="yp")
                       for o in range(2)]
                for s in range(4):
                    for o in range(2):
                        nc.tensor.matmul(
                            pso[o], lhsT=(att[s][:, t * P:(t + 1) * P]),
                            rhs=(wo_sb[s][:, o * GS:(o + 1) * GS]),
                            start=(s == 0), stop=(s == 3))
                for o in range(2):
                    y_sb = ypool.tile([P, GS], BF16, tag=f"ysb{o}",
                                      name=f"ysb{o}")
                    nc.vector.tensor_copy(y_sb, pso[o])
                    nc.scalar.dma_start(
                        out=y[g0 + t * P: g0 + (t + 1) * P,
                              o * GS:(o + 1) * GS],
                        in_=y_sb)

            for it in range(NG + 1):
                gq = it + 2       # group doing qproj+elu (0/1 ran in pass A)
                ga = it           # group doing attn
                gy = it - 1       # group doing y GEMM
                qp_units = [(h, s) for h in range(HPC) for s in range(2)] \
                    if gq < NG else []
                att_y = [atts.pop((gy, hp)) for hp in range(4)] \
                    if 0 <= gy < NG else None
                iq = 0
                for i in range(4):
                    if ga < NG:
                        unit_attn(ga, i)
                    for _ in range(4):
                        if qp_units:
                            h, s = qp_units.pop(0)
                            unit_qproj(gq, h, s, iq, qpps)
                            iq += 1
                    if att_y is not None:
                        unit_y(gy, att_y, i)


def build(n=SEQ):
    # Bacc (not raw Bass): its compile pipeline splits multi-waits into
    # event semaphores (TRN2 allows at most 1 sync wait per instruction).
    nc = bacc.Bacc("TRN2", target_bir_lowering=False, debug=False,
                   enable_asserts=False)
    xT = nc.declare_dram_parameter("xT", [D, n], BF16, isOutput=False)
    wq = nc.declare_dram_parameter("wq", [D, DH], BF16, isOutput=False)
    wk = nc.declare_dram_parameter("wk", [D, DH], BF16, isOutput=False)
    wv = nc.declare_dram_parameter("wv", [D, DH], BF16, isOutput=False)
    proj = nc.declare_dram_parameter("proj", [DH, F], BF16, isOutput=False)
    wout = nc.declare_dram_parameter("wout", [DH, D], BF16, isOutput=False)
    y = nc.declare_dram_parameter("y", [n, D], BF16, isOutput=True)
    with tile.TileContext(nc) as tc:
        _emit(tc, n, xT, wq, wk, wv, proj, wout, y)
    nc.finalize()
    return nc


def make_in_maps(x, w_qkv, proj_matrix, w_out):
    bf = np.float16
    x = np.asarray(x, np.float32)
    w_qkv = np.asarray(w_qkv, bf)
    proj_matrix = np.asarray(proj_matrix, bf)
    w_out = np.asarray(w_out, bf)
    in_maps = []
    for c in range(NCORES):
        b, g = c // 2, c % 2
        in_maps.append({
            "xT": np.ascontiguousarray(x[b].T.astype(bf)),
            "wq": np.ascontiguousarray(w_qkv[:, DH * g:DH * (g + 1)]),
            "wk": np.ascontiguousarray(w_qkv[:, D + DH * g:D + DH * (g + 1)]),
            "wv": np.ascontiguousarray(w_qkv[:, 2 * D + DH * g:2 * D + DH * (g + 1)]),
            "proj": np.ascontiguousarray(
                proj_matrix[HPC * g:HPC * (g + 1)].reshape(DH, F)),
            "wout": np.ascontiguousarray(w_out[DH * g:DH * (g + 1), :]),
        })
    return in_maps


_NC_CACHE = {}


def get_nc(n=SEQ):
    if n not in _NC_CACHE:
        _NC_CACHE[n] = build(n)
    return _NC_CACHE[n]


def _install_ntff_hook_shim():
    """The agent image's antenv lacks axon_hooks; recreate it so
    run_bass_kernel_spmd(trace=True) can capture NTFF profiles."""
    import sys
    import types
    try:
        from antenv.axon_hooks import get_axon_ntff_profile_hook  # noqa: F401
        return True
    except ImportError:
        pass
    try:
        from trn_agent_boot.trn_boot import _ntff_profile_via_ctypes
        import antenv
        mod = types.ModuleType("antenv.axon_hooks")
        mod._hook = _ntff_profile_via_ctypes("/opt/axon/libaxon_pjrt.so")
        mod.set_axon_ntff_profile_hook = lambda h: setattr(mod, "_hook", h)
        mod.get_axon_ntff_profile_hook = lambda: mod._hook
        sys.modules["antenv.axon_hooks"] = mod
        antenv.axon_hooks = mod
        return True
    except Exception as e:  # profiling is best-effort
        print(f"ntff hook shim failed: {e}")
        return False


def run(x, w_qkv, proj_matrix, w_out, b_out, trace=False, **kw):
    if trace:
        _install_ntff_hook_shim()
    nc = get_nc(SEQ)
    in_maps = make_in_maps(x, w_qkv, proj_matrix, w_out)
    res = run_bass_kernel_spmd(nc, in_maps, list(range(NCORES)),
                               trace=trace, **kw)
    b_out = np.asarray(b_out, np.float32)
    out = np.empty((B, SEQ, D), np.float32)
    for b in range(B):
        out[b] = np.asarray(res.results[2 * b]["y"], np.float32) \
            + np.asarray(res.results[2 * b + 1]["y"], np.float32) \
            + b_out[None, :]
    return out, res


def kernel(x, w_qkv, proj_matrix, w_out, b_out):
    out, _ = run(x, w_qkv, proj_matrix, w_out, b_out)
    return out


# revision 12
# speedup vs baseline: 1.0386x; 1.0386x over previous
"""Trainium2 Bass kernel: Performer (linear) attention + in/out projections.

Problem nn_LinearPerformerAttention_6717328851263:
  x:(4,4096,1024) f32, w_qkv:(1024,3072), proj_matrix:(16,64,256),
  w_out:(1024,1024), b_out:(1024,)

  qkv = x @ w_qkv ; split q,k,v ; per (b,h): q_proj=elu1(q@P_h), k_proj=elu1(k@P_h)
  kv = k_proj^T v ; k_sum = sum_n k_proj ; attn = (q_proj @ kv) / (q_proj@k_sum)
  out = attn @ w_out + b_out

Sharding over 8 cores: core c -> (batch b=c//2, head-group g=c%2: 8 of 16 heads).
Each core computes partial y_c = attn(b, heads_g) @ w_out[512g:512g+512, :].
Host gather: out[b] = y_(b,0) + y_(b,1) + b_out.

v1 rewrite vs baseline (594 us):
- all matmul operands bf16 (rel_fro ~3.5e-3 vs 2e-2 gate, CPU-simulated):
  f32r pays 4x cycles on <256-wide streams and the TRN2 PE p-state ramp
  (0.65/1.2/2.4 GHz; max only after 3us of CONTINUOUS execution) punishes
  any stall; bf16 is 1 cycle/row at every width.
- qT kept SBUF-resident for pass B (kills the 16 MiB DRAM spill round trip).
- kv state computed directly in [F,d] orientation (lhsT=k_projE, rhs=v|1):
  65-row streams, ~2x fewer PE cycles than the old [d,F]+transpose fixup,
  and the fixup transposes disappear (ksr is built with one tensor_scalar).
- 3-stage software pipeline in both passes: PE stream for iteration i is
  [independent GEMMs for group g | proj mms for g-1 | consumer mms for g-2]
  so matmuls never wait on the elu chain; PSUM tiles are drained to SBUF
  bf16 right after production (PSUM can only hold ~8 [128,512] tiles).
- elu1(x)=min(exp(x),1)+relu(x): exp on Scalar, relu split Scalar/Vector,
  min+add on Vector.  All elu intermediates stay f32 (DVE ops with 2-byte
  INPUTS hit a ~10x slow path; f32-in/bf16-out runs at full rate), bf16 is
  written only at the matmul-input boundary.  GpSimd runs nothing bulky
  (Q7 tensor routines are ~12x slower than DVE).
"""

import numpy as np
from contextlib import ExitStack

import ml_dtypes

import concourse.bass as bass
import concourse.bacc as bacc
import concourse.tile as tile
from concourse import mybir
from concourse.bass_utils import run_bass_kernel_spmd

FP32 = mybir.dt.float32
BF16 = mybir.dt.float16  # fp16: DVE-native 16-bit (bf16 inputs hit a slow DVE path)
AL = mybir.AluOpType
AF = mybir.ActivationFunctionType

B, SEQ, D = 4, 4096, 1024
H, HD, F = 16, 64, 256
HPC = 8            # heads per core
DH = HPC * HD      # 512 head-space dims per core
P = 128
NCORES = 8
GS = 512           # tokens per group
TPG = 4            # 128-token tiles per group


def _emit(tc, n, xT, wq, wk, wv, proj, wout, y):
    nc = tc.nc
    NG = n // GS

    def copy_op(idx):
        # alternate PSUM->SBUF eviction between Scalar (activation Copy)
        # and Vector engines
        return nc.scalar.copy if idx % 2 == 0 else nc.vector.tensor_copy

    ctx = ExitStack()
    with ctx:
        const = ctx.enter_context(tc.tile_pool(name="const", bufs=1))

        ones_bf = const.tile([P, P], BF16, tag="ones", name="ones")
        nc.vector.memset(ones_bf, 1.0)
        ones_f32 = const.tile([P, HD], FP32, tag="onesf", name="onesf")
        nc.vector.memset(ones_f32, 1.0)

        # proj, pair-packed [128, 256]: head 2i at partitions 0:64, head
        # 2i+1 at 64:128 (lhsT/rhs partition bases always match).
        proj_pair = [const.tile([P, F], BF16, tag=f"projp{i}", name=f"projp{i}")
                     for i in range(4)]
        for i in range(4):
            nc.sync.dma_start(out=proj_pair[i], in_=proj[i * P:(i + 1) * P, :])

        # attn lhsT, zero-padded so a head pair accumulates into one
        # [128,512] PSUM tile: kvS[h][s] [128 F-slab, 128]; cols (h%2)*64..
        # hold kv_h, other 64 cols zero.  ksr[h][s]: same but columns
        # replicate k_sum_h (denominator lands on matching partitions).
        kvS = [[const.tile([P, P], BF16, tag=f"kvS{h}_{s}", name=f"kvS{h}_{s}")
                for s in range(2)] for h in range(HPC)]
        ksr = [[const.tile([P, P], BF16, tag=f"ksr{h}_{s}", name=f"ksr{h}_{s}")
                for s in range(2)] for h in range(HPC)]
        for h in range(HPC):
            for s in range(2):
                nc.gpsimd.memset(kvS[h][s], 0.0)
                nc.gpsimd.memset(ksr[h][s], 0.0)

        # kv accumulator per head: [128 F-sub, 2 s-slabs, 65] f32.
        # col 64 = k_sum (ones column of vone).
        kv_acc = [const.tile([P, 2, HD + 1], FP32, tag=f"kva{h}", name=f"kva{h}")
                  for h in range(HPC)]

        # full-sequence qT, pair-packed [128, 4 pairs, n] fp16 (4 MiB)
        qT_sb = const.tile([P, 4, n], BF16, tag="qTs", name="qTs")

        # pass-B qproj elu pools live at ctx level: the last two pass-A
        # iterations (PE nearly idle) already emit qproj for groups 0/1
        qelupool = ctx.enter_context(tc.tile_pool(name="qelupool", bufs=4))
        qppool = ctx.enter_context(tc.tile_pool(name="qppool", bufs=3))
        qPs = {}   # (g, h, s) -> qP tile fp16

        def unit_qproj(g, h, s, idx, psum_pool, ptag="qp"):
            g0 = g * GS
            hp, hb = h // 2, (h % 2) * HD
            qp = psum_pool.tile([P, GS], FP32, tag=ptag, name=ptag)
            nc.tensor.matmul(
                qp, lhsT=(proj_pair[hp][hb:hb + HD, s * P:(s + 1) * P]),
                rhs=(qT_sb[hb:hb + HD, hp, g0:g0 + GS]),
                start=True, stop=True)
            qE = qelupool.tile([P, GS], BF16, tag="qE", name="qE")
            qR = qelupool.tile([P, GS], BF16, tag="qR", name="qR")
            nc.scalar.activation(qE, qp, AF.Exp)
            if idx % 2 == 0:
                nc.scalar.activation(qR, qp, AF.Relu)
            else:
                nc.vector.tensor_scalar_max(qR, qp, 0.0)
            qP = qppool.tile([P, GS], BF16, tag=f"qP{h}_{s}",
                             name=f"qP{h}_{s}")
            nc.vector.scalar_tensor_tensor(
                qP, in0=qE, scalar=1.0, in1=qR, op0=AL.min, op1=AL.add)
            qPs[(g, h, s)] = qP

        # ---------------- pass A ----------------
        # 3-stage software pipeline; within an iteration the PE stream
        # interleaves group g's qkv GEMM units with group g-1's kproj units
        # (so elu1 chases a slow trickle of PSUM tiles instead of a burst)
        # and ends with group g-2's kv matmuls (whose kP inputs got a full
        # iteration of elu latency).
        with ExitStack() as actx:
            wpool = actx.enter_context(tc.tile_pool(name="wpool", bufs=1))
            wq_sb = [wpool.tile([P, DH], BF16, tag=f"wq{s}", name=f"wq{s}") for s in range(8)]
            wk_sb = [wpool.tile([P, DH], BF16, tag=f"wk{s}", name=f"wk{s}") for s in range(8)]
            wv_sb = [wpool.tile([P, DH], BF16, tag=f"wv{s}", name=f"wv{s}") for s in range(8)]

            xtpool = actx.enter_context(tc.tile_pool(name="xtpool", bufs=2))
            ktpool = actx.enter_context(tc.tile_pool(name="ktpool", bufs=2))
            vpool = actx.enter_context(tc.tile_pool(name="vpool", bufs=3))
            elupool = actx.enter_context(tc.tile_pool(name="elupool", bufs=4))
            kppool = actx.enter_context(tc.tile_pool(name="kppool", bufs=2))
            mmps = actx.enter_context(tc.tile_pool(name="mmps", bufs=2, space="PSUM"))
            cpps = actx.enter_context(tc.tile_pool(name="cpps", bufs=4, space="PSUM"))
            kvps = actx.enter_context(tc.tile_pool(name="kvps", bufs=2, space="PSUM"))

            xT_v = xT.rearrange("(s p) m -> p s m", p=P)

            kts = {}    # g -> kt tile [128, 4, 512]
            vones = {}  # g -> vone tile [128, 4, 8, 65]
            kPs = {}    # (g, hp, tp, h) -> kP tile [128, 512] fp16
            xts = {}    # g -> xt tile [128, 8, 512]

            def fetch_xt(g):
                # per-slab sub-DMAs across the three DMA queues: the first
                # matmul only has to wait for slab 0, and queues run parallel
                g0 = g * GS
                xt = xtpool.tile([P, 8, GS], BF16, tag="xt", name="xt")
                qs = [nc.sync, nc.scalar, nc.gpsimd]
                for s in range(8):
                    qs[s % 3].dma_start(out=xt[:, s, :],
                                        in_=xT_v[:, s, g0:g0 + GS])
                xts[g] = xt

            def unit_xt(g):
                xt = xts.pop(g)
                kts[g] = ktpool.tile([P, 4, GS], BF16, tag="kt", name="kt")
                vone = vpool.tile([P, TPG, HPC, HD + 1], BF16, tag="vone",
                                  name="vone")
                nc.vector.tensor_copy(
                    vone[:, :, :, HD],
                    ones_bf[:, 0:TPG * HPC].rearrange("p (t h) -> p t h", t=TPG))
                vones[g] = vone
                return xt

            def unit_q(g, xt, fs):
                g0 = g * GS
                ps = mmps.tile([P, GS], FP32, tag="mm", name="mm")
                for s in range(8):
                    nc.tensor.matmul(
                        ps, lhsT=(wq_sb[s][:, fs * P:(fs + 1) * P]),
                        rhs=(xt[:, s, :]), start=(s == 0), stop=(s == 7))
                copy_op(fs)(qT_sb[:, fs, g0:g0 + GS], ps)

            def unit_k(g, xt, fs):
                ps = mmps.tile([P, GS], FP32, tag="mm", name="mm")
                for s in range(8):
                    nc.tensor.matmul(
                        ps, lhsT=(wk_sb[s][:, fs * P:(fs + 1) * P]),
                        rhs=(xt[:, s, :]), start=(s == 0), stop=(s == 7))
                copy_op(fs + 1)(kts[g][:, fs, :], ps)

            def unit_v(g, xt, t):
                ps = mmps.tile([P, GS], FP32, tag="mm", name="mm")
                for s in range(8):
                    nc.tensor.matmul(
                        ps, lhsT=(xt[:, s, t * P:(t + 1) * P]),
                        rhs=(wv_sb[s]), start=(s == 0), stop=(s == 7))
                copy_op(t)(
                    vones[g][:, t, :, 0:HD],
                    ps.rearrange("p (h e) -> p h e", h=HPC))

            def unit_kproj(g, hp, tp, h, idx):
                # c[tokens, 2ti x 256F] = k_h @ P_h, then elu1 -> kP bf16.
                # Even/odd heads sit at base partitions 0/64 so the PE runs
                # them in disjoint row groups.
                hb = (h % 2) * HD
                kt = kts[g]
                c = cpps.tile([P, GS], FP32, tag="kc", name="kc")
                for ti in range(2):
                    t = tp * 2 + ti
                    nc.tensor.matmul(
                        c[:, ti * F:(ti + 1) * F],
                        lhsT=(kt[hb:hb + HD, hp, t * P:(t + 1) * P]),
                        rhs=(proj_pair[hp][hb:hb + HD, :]),
                        start=True, stop=True)
                kE = elupool.tile([P, GS], BF16, tag="kE", name="kE")
                kR = elupool.tile([P, GS], BF16, tag="kR", name="kR")
                nc.scalar.activation(kE, c, AF.Exp)
                if idx % 16 < 10:  # balance Scalar vs Vector load
                    nc.scalar.activation(kR, c, AF.Relu)
                else:
                    nc.vector.tensor_scalar_max(kR, c, 0.0)
                kP = kppool.tile([P, GS], BF16, tag=f"kP{hp}_{tp}_{h % 2}",
                                 name=f"kP{hp}_{tp}_{h % 2}")
                nc.vector.scalar_tensor_tensor(
                    kP, in0=kE, scalar=1.0, in1=kR, op0=AL.min, op1=AL.add)
                kPs[(g, hp, tp, h)] = kP

            def unit_kv(g, hp, h):
                # kv[f, d] += k_projE^T [v|1] per (head, F-slab), PSUM
                # accumulated over the 4 token tiles, folded into kv_acc f32.
                vone = vones[g]
                kv_ps = kvps.tile([P, 2, P], FP32, tag="kv", name="kv")
                for s in range(2):
                    for t in range(TPG):
                        tp, ti = t // 2, t % 2
                        kP = kPs.pop((g, hp, tp, h)) if s == 1 and t == TPG - 1 \
                            else kPs[(g, hp, tp, h)]
                        nc.tensor.matmul(
                            kv_ps[:, s, 0:HD + 1],
                            lhsT=(kP[:, ti * F + s * P: ti * F + s * P + P]),
                            rhs=(vone[:, t, h, :]),
                            start=(t == 0), stop=(t == TPG - 1),
                            skip_group_check=True)
                if g == 0:
                    nc.vector.tensor_copy(kv_acc[h], kv_ps[:, :, 0:HD + 1])
                else:
                    nc.vector.tensor_tensor(
                        out=kv_acc[h], in0=kv_ps[:, :, 0:HD + 1],
                        in1=kv_acc[h], op=AL.add)

            fetch_xt(0)  # x tile 0 in flight before the weight loads
            for s in range(8):
                nc.scalar.dma_start(out=wq_sb[s], in_=wq[s * P:(s + 1) * P, :])
                nc.gpsimd.dma_start(out=wk_sb[s], in_=wk[s * P:(s + 1) * P, :])
                (nc.scalar if s % 2 else nc.gpsimd).dma_start(
                    out=wv_sb[s], in_=wv[s * P:(s + 1) * P, :])

            for it in range(NG + 2):
                g = it            # group doing qkv GEMMs
                gk = it - 1       # group doing kproj+elu
                gv = it - 2       # group doing kv accumulation
                xt = unit_xt(g) if g < NG else None
                # 12 qkv units interleaved with 16 kproj units
                kp_units = []
                if 0 <= gk < NG:
                    kp_units = [(hp, tp, h)
                                for hp in range(HPC // 2)
                                for tp in range(2)
                                for h in (2 * hp, 2 * hp + 1)]
                qkv_units = []
                if g < NG:
                    qkv_units = [("q", fs) for fs in range(4)] + \
                                [("k", fs) for fs in range(4)] + \
                                [("v", t) for t in range(4)]
                ik = 0
                for iu, u in enumerate(qkv_units):
                    kind, a = u
                    if kind == "q":
                        unit_q(g, xt, a)
                    elif kind == "k":
                        unit_k(g, xt, a)
                    else:
                        unit_v(g, xt, a)
                    if iu == 3 and g + 1 < NG:
                        fetch_xt(g + 1)
                    # ~16 kproj units spread over 12 qkv units
                    while len(kp_units) * 12 > (11 - iu) * 16 and kp_units:
                        hp, tp, h = kp_units.pop(0)
                        unit_kproj(gk, hp, tp, h, ik)
                        ik += 1
                for hp, tp, h in kp_units:
                    unit_kproj(gk, hp, tp, h, ik)
                    ik += 1
                if 0 <= gv < NG:
                    for hp in range(HPC // 2):
                        for h in (2 * hp, 2 * hp + 1):
                            unit_kv(gv, hp, h)
                # tail iterations: PE is nearly idle, pre-run pass-B qproj
                # for groups 0/1 out of the mmps psum pool
                if it >= NG:
                    gq = it - NG
                    for iq, (h, s) in enumerate(
                            [(h, s) for h in range(HPC) for s in range(2)]):
                        unit_qproj(gq, h, s, iq, mmps, ptag="mm")

        # ---------------- kv fixup: kv_acc -> kvS/ksr (bf16) ----------------
        for h in range(HPC):
            hb = (h % 2) * HD
            for s in range(2):
                nc.vector.tensor_copy(kvS[h][s][:, hb:hb + HD],
                                      kv_acc[h][:, s, 0:HD])
                # ksr cols hb:hb+64 = k_sum[f] broadcast along free dim
                # (scalar engine: copy of ones scaled per-partition by k_sum)
                nc.scalar.activation(
                    ksr[h][s][:, hb:hb + HD], ones_f32, AF.Copy,
                    scale=kv_acc[h][:, s, HD:HD + 1])

        # ---------------- pass B ----------------
        # same 3-stage pipeline; qproj units trickle between attn/y units.
        with ExitStack() as bctx:
            wopool = bctx.enter_context(tc.tile_pool(name="wopool", bufs=1))
            wo_sb = [wopool.tile([P, D], BF16, tag=f"wo{s}", name=f"wo{s}") for s in range(4)]
            for s in range(4):
                nc.sync.dma_start(out=wo_sb[s], in_=wout[s * P:(s + 1) * P, :])

            attpool = bctx.enter_context(tc.tile_pool(name="attpool", bufs=3))
            zpool = bctx.enter_context(tc.tile_pool(name="zpool", bufs=2))
            ypool = bctx.enter_context(tc.tile_pool(name="ypool", bufs=2))
            qpps = bctx.enter_context(tc.tile_pool(name="qpps", bufs=2, space="PSUM"))
            atps = bctx.enter_context(tc.tile_pool(name="atps", bufs=2, space="PSUM"))
            dnps = bctx.enter_context(tc.tile_pool(name="dnps", bufs=2, space="PSUM"))
            yps = bctx.enter_context(tc.tile_pool(name="yps", bufs=2, space="PSUM"))

            atts = {}  # (g, hp) -> att_sb tile fp16

            def unit_attn(g, hp):
                aps = atps.tile([P, GS], FP32, tag="at", name="at")
                dps = dnps.tile([P, GS], FP32, tag="dn", name="dn")
                for h in (2 * hp, 2 * hp + 1):
                    first = h % 2 == 0
                    last = h % 2 == 1
                    for s in range(2):
                        qP = qPs.pop((g, h, s)) if last and s == 1 \
                            else qPs[(g, h, s)]
                        nc.tensor.matmul(
                            aps, lhsT=(kvS[h][s]), rhs=(qP),
                            start=(first and s == 0), stop=(last and s == 1),
                            skip_group_check=True)
                        nc.tensor.matmul(
                            dps, lhsT=(ksr[h][s]), rhs=(qP),
                            start=(first and s == 0), stop=(last and s == 1),
                            skip_group_check=True)
                zb = zpool.tile([P, GS], FP32, tag="zb", name="zb")
                nc.vector.reciprocal_approx_fast(zb, dps)
                att = attpool.tile([P, GS], BF16, tag=f"att{hp}",
                                   name=f"att{hp}")
                nc.vector.tensor_tensor(out=att, in0=aps, in1=zb, op=AL.mult)
                atts[(g, hp)] = att

            def unit_y(g, att, t):
                g0 = g * GS
                pso = [yps.tile([P, GS], FP32, tag="yp", name="yp")
                       for o in range(2)]
                for s in range(4):
                    for o in range(2):
                        nc.tensor.matmul(
                            pso[o], lhsT=(att[s][:, t * P:(t + 1) * P]),
                            rhs=(wo_sb[s][:, o * GS:(o + 1) * GS]),
                            start=(s == 0), stop=(s == 3))
                for o in range(2):
                    y_sb = ypool.tile([P, GS], BF16, tag=f"ysb{o}",
                                      name=f"ysb{o}")
                    nc.scalar.copy(y_sb, pso[o])
                    nc.scalar.dma_start(
                        out=y[g0 + t * P: g0 + (t + 1) * P,
                              o * GS:(o + 1) * GS],
                        in_=y_sb)

            for it in range(NG + 1):
                gq = it + 2       # group doing qproj+elu (0/1 ran in pass A)
                ga = it           # group doing attn
                gy = it - 1       # group doing y GEMM
                qp_units = [(h, s) for h in range(HPC) for s in range(2)] \
                    if gq < NG else []
                att_y = [atts.pop((gy, hp)) for hp in range(4)] \
                    if 0 <= gy < NG else None
                iq = 0
                for i in range(4):
                    if ga < NG:
                        unit_attn(ga, i)
                    for _ in range(4):
                        if qp_units:
                            h, s = qp_units.pop(0)
                            unit_qproj(gq, h, s, iq, qpps)
                            iq += 1
                    if att_y is not None:
                        unit_y(gy, att_y, i)


def build(n=SEQ):
    # Bacc (not raw Bass): its compile pipeline splits multi-waits into
    # event semaphores (TRN2 allows at most 1 sync wait per instruction).
    nc = bacc.Bacc("TRN2", target_bir_lowering=False, debug=False,
                   enable_asserts=False)
    xT = nc.declare_dram_parameter("xT", [D, n], BF16, isOutput=False)
    wq = nc.declare_dram_parameter("wq", [D, DH], BF16, isOutput=False)
    wk = nc.declare_dram_parameter("wk", [D, DH], BF16, isOutput=False)
    wv = nc.declare_dram_parameter("wv", [D, DH], BF16, isOutput=False)
    proj = nc.declare_dram_parameter("proj", [DH, F], BF16, isOutput=False)
    wout = nc.declare_dram_parameter("wout", [DH, D], BF16, isOutput=False)
    y = nc.declare_dram_parameter("y", [n, D], BF16, isOutput=True)
    with tile.TileContext(nc) as tc:
        _emit(tc, n, xT, wq, wk, wv, proj, wout, y)
    nc.finalize()
    return nc


def make_in_maps(x, w_qkv, proj_matrix, w_out):
    bf = np.float16
    x = np.asarray(x, np.float32)
    w_qkv = np.asarray(w_qkv, bf)
    proj_matrix = np.asarray(proj_matrix, bf)
    w_out = np.asarray(w_out, bf)
    in_maps = []
    for c in range(NCORES):
        b, g = c // 2, c % 2
        in_maps.append({
            "xT": np.ascontiguousarray(x[b].T.astype(bf)),
            "wq": np.ascontiguousarray(w_qkv[:, DH * g:DH * (g + 1)]),
            "wk": np.ascontiguousarray(w_qkv[:, D + DH * g:D + DH * (g + 1)]),
            "wv": np.ascontiguousarray(w_qkv[:, 2 * D + DH * g:2 * D + DH * (g + 1)]),
            "proj": np.ascontiguousarray(
                proj_matrix[HPC * g:HPC * (g + 1)].reshape(DH, F)),
            "wout": np.ascontiguousarray(w_out[DH * g:DH * (g + 1), :]),
        })
    return in_maps


_NC_CACHE = {}


def get_nc(n=SEQ):
    if n not in _NC_CACHE:
        _NC_CACHE[n] = build(n)
    return _NC_CACHE[n]


def _install_ntff_hook_shim():
    """The agent image's antenv lacks axon_hooks; recreate it so
    run_bass_kernel_spmd(trace=True) can capture NTFF profiles."""
    import sys
    import types
    try:
        from antenv.axon_hooks import get_axon_ntff_profile_hook  # noqa: F401
        return True
    except ImportError:
        pass
    try:
        from trn_agent_boot.trn_boot import _ntff_profile_via_ctypes
        import antenv
        mod = types.ModuleType("antenv.axon_hooks")
        mod._hook = _ntff_profile_via_ctypes("/opt/axon/libaxon_pjrt.so")
        mod.set_axon_ntff_profile_hook = lambda h: setattr(mod, "_hook", h)
        mod.get_axon_ntff_profile_hook = lambda: mod._hook
        sys.modules["antenv.axon_hooks"] = mod
        antenv.axon_hooks = mod
        return True
    except Exception as e:  # profiling is best-effort
        print(f"ntff hook shim failed: {e}")
        return False


def run(x, w_qkv, proj_matrix, w_out, b_out, trace=False, **kw):
    if trace:
        _install_ntff_hook_shim()
    nc = get_nc(SEQ)
    in_maps = make_in_maps(x, w_qkv, proj_matrix, w_out)
    res = run_bass_kernel_spmd(nc, in_maps, list(range(NCORES)),
                               trace=trace, **kw)
    b_out = np.asarray(b_out, np.float32)
    out = np.empty((B, SEQ, D), np.float32)
    for b in range(B):
        out[b] = np.asarray(res.results[2 * b]["y"], np.float32) \
            + np.asarray(res.results[2 * b + 1]["y"], np.float32) \
            + b_out[None, :]
    return out, res


def kernel(x, w_qkv, proj_matrix, w_out, b_out):
    out, _ = run(x, w_qkv, proj_matrix, w_out, b_out)
    return out


# revision 13
# speedup vs baseline: 1.0660x; 1.0264x over previous
"""Trainium2 Bass kernel: Performer (linear) attention + in/out projections.

Problem nn_LinearPerformerAttention_6717328851263:
  x:(4,4096,1024) f32, w_qkv:(1024,3072), proj_matrix:(16,64,256),
  w_out:(1024,1024), b_out:(1024,)

  qkv = x @ w_qkv ; split q,k,v ; per (b,h): q_proj=elu1(q@P_h), k_proj=elu1(k@P_h)
  kv = k_proj^T v ; k_sum = sum_n k_proj ; attn = (q_proj @ kv) / (q_proj@k_sum)
  out = attn @ w_out + b_out

Sharding over 8 cores: core c -> (batch b=c//2, head-group g=c%2: 8 of 16 heads).
Each core computes partial y_c = attn(b, heads_g) @ w_out[512g:512g+512, :].
Host gather: out[b] = y_(b,0) + y_(b,1) + b_out.

v1 rewrite vs baseline (594 us):
- all matmul operands bf16 (rel_fro ~3.5e-3 vs 2e-2 gate, CPU-simulated):
  f32r pays 4x cycles on <256-wide streams and the TRN2 PE p-state ramp
  (0.65/1.2/2.4 GHz; max only after 3us of CONTINUOUS execution) punishes
  any stall; bf16 is 1 cycle/row at every width.
- qT kept SBUF-resident for pass B (kills the 16 MiB DRAM spill round trip).
- kv state computed directly in [F,d] orientation (lhsT=k_projE, rhs=v|1):
  65-row streams, ~2x fewer PE cycles than the old [d,F]+transpose fixup,
  and the fixup transposes disappear (ksr is built with one tensor_scalar).
- 3-stage software pipeline in both passes: PE stream for iteration i is
  [independent GEMMs for group g | proj mms for g-1 | consumer mms for g-2]
  so matmuls never wait on the elu chain; PSUM tiles are drained to SBUF
  bf16 right after production (PSUM can only hold ~8 [128,512] tiles).
- elu1(x)=min(exp(x),1)+relu(x): exp on Scalar, relu split Scalar/Vector,
  min+add on Vector.  All elu intermediates stay f32 (DVE ops with 2-byte
  INPUTS hit a ~10x slow path; f32-in/bf16-out runs at full rate), bf16 is
  written only at the matmul-input boundary.  GpSimd runs nothing bulky
  (Q7 tensor routines are ~12x slower than DVE).
"""

import numpy as np
from contextlib import ExitStack

import ml_dtypes

import concourse.bass as bass
import concourse.bacc as bacc
import concourse.tile as tile
from concourse import mybir
from concourse.bass_utils import run_bass_kernel_spmd

FP32 = mybir.dt.float32
BF16 = mybir.dt.float16  # fp16: DVE-native 16-bit (bf16 inputs hit a slow DVE path)
AL = mybir.AluOpType
AF = mybir.ActivationFunctionType

B, SEQ, D = 4, 4096, 1024
H, HD, F = 16, 64, 256
HPC = 8            # heads per core
DH = HPC * HD      # 512 head-space dims per core
P = 128
NCORES = 8
GS = 512           # tokens per group
TPG = 4            # 128-token tiles per group


def _emit(tc, n, xT, wq, wk, wv, proj, wout, y, qPd):
    nc = tc.nc
    NG = n // GS
    SPILL_G = 4       # groups whose qproj+elu runs in pass A (qP via DRAM)

    def copy_op(idx):
        # alternate PSUM->SBUF eviction between Scalar (activation Copy)
        # and Vector engines
        return nc.scalar.copy if idx % 2 == 0 else nc.vector.tensor_copy

    ctx = ExitStack()
    with ctx:
        const = ctx.enter_context(tc.tile_pool(name="const", bufs=1))

        ones_bf = const.tile([P, P], BF16, tag="ones", name="ones")
        nc.vector.memset(ones_bf, 1.0)
        ones_f32 = const.tile([P, HD], FP32, tag="onesf", name="onesf")
        nc.vector.memset(ones_f32, 1.0)

        # proj, pair-packed [128, 256]: head 2i at partitions 0:64, head
        # 2i+1 at 64:128 (lhsT/rhs partition bases always match).
        proj_pair = [const.tile([P, F], BF16, tag=f"projp{i}", name=f"projp{i}")
                     for i in range(4)]
        for i in range(4):
            nc.sync.dma_start(out=proj_pair[i], in_=proj[i * P:(i + 1) * P, :])

        # attn lhsT, zero-padded so a head pair accumulates into one
        # [128,512] PSUM tile: kvS[h][s] [128 F-slab, 128]; cols (h%2)*64..
        # hold kv_h, other 64 cols zero.  ksr[h][s]: same but columns
        # replicate k_sum_h (denominator lands on matching partitions).
        kvS = [[const.tile([P, P], BF16, tag=f"kvS{h}_{s}", name=f"kvS{h}_{s}")
                for s in range(2)] for h in range(HPC)]
        ksr = [[const.tile([P, P], BF16, tag=f"ksr{h}_{s}", name=f"ksr{h}_{s}")
                for s in range(2)] for h in range(HPC)]
        for h in range(HPC):
            for s in range(2):
                nc.gpsimd.memset(kvS[h][s], 0.0)
                nc.gpsimd.memset(ksr[h][s], 0.0)

        # kv accumulator per head: [128 F-sub, 2 s-slabs, 65] f32.
        # col 64 = k_sum (ones column of vone).
        kv_acc = [const.tile([P, 2, HD + 1], FP32, tag=f"kva{h}", name=f"kva{h}")
                  for h in range(HPC)]

        # full-sequence qT, pair-packed [128, 4 pairs, n] fp16 (4 MiB)
        qT_sb = const.tile([P, 4, n], BF16, tag="qTs", name="qTs")

        # pass-B qproj elu pools live at ctx level: the last two pass-A
        # iterations (PE nearly idle) already emit qproj for groups 0/1
        qelupool = ctx.enter_context(tc.tile_pool(name="qelupool", bufs=4))
        qppool = ctx.enter_context(tc.tile_pool(name="qppool", bufs=3))
        qPs = {}   # (g, h, s) -> qP tile fp16

        def unit_qproj(g, h, s, idx, psum_pool, ptag="qp", spill=False):
            g0 = g * GS
            hp, hb = h // 2, (h % 2) * HD
            qp = psum_pool.tile([P, GS], FP32, tag=ptag, name=ptag)
            nc.tensor.matmul(
                qp, lhsT=(proj_pair[hp][hb:hb + HD, s * P:(s + 1) * P]),
                rhs=(qT_sb[hb:hb + HD, hp, g0:g0 + GS]),
                start=True, stop=True)
            qE = qelupool.tile([P, GS], BF16, tag="qE", name="qE")
            qR = qelupool.tile([P, GS], BF16, tag="qR", name="qR")
            nc.scalar.activation(qE, qp, AF.Exp)
            if idx % 2 == 0:
                nc.scalar.activation(qR, qp, AF.Relu)
            else:
                nc.vector.tensor_scalar_max(qR, qp, 0.0)
            qP = qppool.tile([P, GS], BF16, tag=f"qP{h}_{s}",
                             name=f"qP{h}_{s}")
            nc.vector.scalar_tensor_tensor(
                qP, in0=qE, scalar=1.0, in1=qR, op0=AL.min, op1=AL.add)
            if spill:
                r0 = (2 * h + s) * P
                nc.sync.dma_start(out=qPd[r0:r0 + P, g0:g0 + GS], in_=qP)
            else:
                qPs[(g, h, s)] = qP

        def readback_qP(g):
            g0 = g * GS
            for h in range(HPC):
                for s in range(2):
                    qP = qppool.tile([P, GS], BF16, tag=f"qP{h}_{s}",
                                     name=f"qP{h}_{s}")
                    r0 = (2 * h + s) * P
                    nc.sync.dma_start(out=qP, in_=qPd[r0:r0 + P, g0:g0 + GS])
                    qPs[(g, h, s)] = qP

        # ---------------- pass A ----------------
        # 3-stage software pipeline; within an iteration the PE stream
        # interleaves group g's qkv GEMM units with group g-1's kproj units
        # (so elu1 chases a slow trickle of PSUM tiles instead of a burst)
        # and ends with group g-2's kv matmuls (whose kP inputs got a full
        # iteration of elu latency).
        with ExitStack() as actx:
            wpool = actx.enter_context(tc.tile_pool(name="wpool", bufs=1))
            wq_sb = [wpool.tile([P, DH], BF16, tag=f"wq{s}", name=f"wq{s}") for s in range(8)]
            wk_sb = [wpool.tile([P, DH], BF16, tag=f"wk{s}", name=f"wk{s}") for s in range(8)]
            wv_sb = [wpool.tile([P, DH], BF16, tag=f"wv{s}", name=f"wv{s}") for s in range(8)]

            xtpool = actx.enter_context(tc.tile_pool(name="xtpool", bufs=2))
            ktpool = actx.enter_context(tc.tile_pool(name="ktpool", bufs=2))
            vpool = actx.enter_context(tc.tile_pool(name="vpool", bufs=3))
            elupool = actx.enter_context(tc.tile_pool(name="elupool", bufs=6))
            kppool = actx.enter_context(tc.tile_pool(name="kppool", bufs=2))
            mmps = actx.enter_context(tc.tile_pool(name="mmps", bufs=2, space="PSUM"))
            cpps = actx.enter_context(tc.tile_pool(name="cpps", bufs=4, space="PSUM"))
            kvps = actx.enter_context(tc.tile_pool(name="kvps", bufs=2, space="PSUM"))

            xT_v = xT.rearrange("(s p) m -> p s m", p=P)

            kts = {}    # g -> kt tile [128, 4, 512]
            vones = {}  # g -> vone tile [128, 4, 8, 65]
            kPs = {}    # (g, hp, tp, h) -> kP tile [128, 512] fp16
            xts = {}    # g -> xt tile [128, 8, 512]

            def fetch_xt(g):
                # per-slab sub-DMAs across the three DMA queues: the first
                # matmul only has to wait for slab 0, and queues run parallel
                g0 = g * GS
                xt = xtpool.tile([P, 8, GS], BF16, tag="xt", name="xt")
                qs = [nc.sync, nc.scalar, nc.gpsimd]
                for s in range(8):
                    qs[s % 3].dma_start(out=xt[:, s, :],
                                        in_=xT_v[:, s, g0:g0 + GS])
                xts[g] = xt

            def unit_xt(g):
                xt = xts.pop(g)
                kts[g] = ktpool.tile([P, 4, GS], BF16, tag="kt", name="kt")
                vone = vpool.tile([P, TPG, HPC, HD + 1], BF16, tag="vone",
                                  name="vone")
                nc.vector.tensor_copy(
                    vone[:, :, :, HD],
                    ones_bf[:, 0:TPG * HPC].rearrange("p (t h) -> p t h", t=TPG))
                vones[g] = vone
                return xt

            def unit_q(g, xt, fs):
                g0 = g * GS
                ps = mmps.tile([P, GS], FP32, tag="mm", name="mm")
                for s in range(8):
                    nc.tensor.matmul(
                        ps, lhsT=(wq_sb[s][:, fs * P:(fs + 1) * P]),
                        rhs=(xt[:, s, :]), start=(s == 0), stop=(s == 7))
                copy_op(fs)(qT_sb[:, fs, g0:g0 + GS], ps)

            def unit_k(g, xt, fs):
                ps = mmps.tile([P, GS], FP32, tag="mm", name="mm")
                for s in range(8):
                    nc.tensor.matmul(
                        ps, lhsT=(wk_sb[s][:, fs * P:(fs + 1) * P]),
                        rhs=(xt[:, s, :]), start=(s == 0), stop=(s == 7))
                copy_op(fs + 1)(kts[g][:, fs, :], ps)

            def unit_v(g, xt, t):
                ps = mmps.tile([P, GS], FP32, tag="mm", name="mm")
                for s in range(8):
                    nc.tensor.matmul(
                        ps, lhsT=(xt[:, s, t * P:(t + 1) * P]),
                        rhs=(wv_sb[s]), start=(s == 0), stop=(s == 7))
                copy_op(t)(
                    vones[g][:, t, :, 0:HD],
                    ps.rearrange("p (h e) -> p h e", h=HPC))

            def unit_kproj(g, hp, tp, h, idx):
                # c[tokens, 2ti x 256F] = k_h @ P_h, then elu1 -> kP bf16.
                # Even/odd heads sit at base partitions 0/64 so the PE runs
                # them in disjoint row groups.
                hb = (h % 2) * HD
                kt = kts[g]
                c = cpps.tile([P, GS], FP32, tag="kc", name="kc")
                for ti in range(2):
                    t = tp * 2 + ti
                    nc.tensor.matmul(
                        c[:, ti * F:(ti + 1) * F],
                        lhsT=(kt[hb:hb + HD, hp, t * P:(t + 1) * P]),
                        rhs=(proj_pair[hp][hb:hb + HD, :]),
                        start=True, stop=True)
                kE = elupool.tile([P, GS], BF16, tag="kE", name="kE")
                kR = elupool.tile([P, GS], BF16, tag="kR", name="kR")
                nc.scalar.activation(kE, c, AF.Exp)
                if idx % 16 < 10:  # balance Scalar vs Vector load
                    nc.scalar.activation(kR, c, AF.Relu)
                else:
                    nc.vector.tensor_scalar_max(kR, c, 0.0)
                kP = kppool.tile([P, GS], BF16, tag=f"kP{hp}_{tp}_{h % 2}",
                                 name=f"kP{hp}_{tp}_{h % 2}")
                nc.vector.scalar_tensor_tensor(
                    kP, in0=kE, scalar=1.0, in1=kR, op0=AL.min, op1=AL.add)
                kPs[(g, hp, tp, h)] = kP

            def unit_kv(g, hp, h):
                # kv[f, d] += k_projE^T [v|1] per (head, F-slab), PSUM
                # accumulated over the 4 token tiles, folded into kv_acc f32.
                vone = vones[g]
                kv_ps = kvps.tile([P, 2, P], FP32, tag="kv", name="kv")
                for s in range(2):
                    for t in range(TPG):
                        tp, ti = t // 2, t % 2
                        kP = kPs.pop((g, hp, tp, h)) if s == 1 and t == TPG - 1 \
                            else kPs[(g, hp, tp, h)]
                        nc.tensor.matmul(
                            kv_ps[:, s, 0:HD + 1],
                            lhsT=(kP[:, ti * F + s * P: ti * F + s * P + P]),
                            rhs=(vone[:, t, h, :]),
                            start=(t == 0), stop=(t == TPG - 1),
                            skip_group_check=True)
                if g == 0:
                    nc.vector.tensor_copy(kv_acc[h], kv_ps[:, :, 0:HD + 1])
                else:
                    nc.vector.tensor_tensor(
                        out=kv_acc[h], in0=kv_ps[:, :, 0:HD + 1],
                        in1=kv_acc[h], op=AL.add)

            fetch_xt(0)  # x tile 0 in flight before the weight loads
            for s in range(8):
                nc.scalar.dma_start(out=wq_sb[s], in_=wq[s * P:(s + 1) * P, :])
                nc.gpsimd.dma_start(out=wk_sb[s], in_=wk[s * P:(s + 1) * P, :])
                (nc.scalar if s % 2 else nc.gpsimd).dma_start(
                    out=wv_sb[s], in_=wv[s * P:(s + 1) * P, :])

            for it in range(NG + 2):
                g = it            # group doing qkv GEMMs
                gk = it - 1       # group doing kproj+elu
                gv = it - 2       # group doing kv accumulation
                xt = unit_xt(g) if g < NG else None
                # proj-side units for this iteration: 16 kproj units for
                # group gk, plus (mid iterations) 16 spill-qproj units for
                # group it-2 -- both share the kc psum rotation and get
                # interleaved among the 12 qkv GEMM units
                mid_units = []
                if 0 <= gk < NG:
                    mid_units += [("kp", hp, tp, h)
                                  for hp in range(HPC // 2)
                                  for tp in range(2)
                                  for h in (2 * hp, 2 * hp + 1)]
                if 2 <= it < 2 + SPILL_G:
                    gq = it - 2
                    qsp = [("qp", gq, h, s)
                           for h in range(HPC) for s in range(2)]
                    # interleave the two kinds evenly
                    mix = []
                    while mid_units or qsp:
                        if mid_units:
                            mix.append(mid_units.pop(0))
                        if qsp:
                            mix.append(qsp.pop(0))
                    mid_units = mix
                n_mid = len(mid_units)
                qkv_units = []
                if g < NG:
                    qkv_units = [("q", fs) for fs in range(4)] + \
                                [("k", fs) for fs in range(4)] + \
                                [("v", t) for t in range(4)]
                ik = 0
                iqp = 0

                def pop_mid():
                    nonlocal ik, iqp
                    u = mid_units.pop(0)
                    if u[0] == "kp":
                        _, hp, tp, h = u
                        unit_kproj(gk, hp, tp, h, ik)
                        ik += 1
                    else:
                        _, gq, h, s = u
                        unit_qproj(gq, h, s, iqp, cpps, ptag="kc",
                                   spill=True)
                        iqp += 1

                for iu, u in enumerate(qkv_units):
                    kind, a = u
                    if kind == "q":
                        unit_q(g, xt, a)
                    elif kind == "k":
                        unit_k(g, xt, a)
                    else:
                        unit_v(g, xt, a)
                    if iu == 3 and g + 1 < NG:
                        fetch_xt(g + 1)
                    while len(mid_units) * 12 > (11 - iu) * n_mid \
                            and mid_units:
                        pop_mid()
                while mid_units:
                    pop_mid()
                if 0 <= gv < NG:
                    for hp in range(HPC // 2):
                        for h in (2 * hp, 2 * hp + 1):
                            unit_kv(gv, hp, h)

        # ---------------- kv fixup: kv_acc -> kvS/ksr (bf16) ----------------
        for h in range(HPC):
            hb = (h % 2) * HD
            for s in range(2):
                nc.vector.tensor_copy(kvS[h][s][:, hb:hb + HD],
                                      kv_acc[h][:, s, 0:HD])
                # ksr cols hb:hb+64 = k_sum[f] broadcast along free dim
                # (scalar engine: copy of ones scaled per-partition by k_sum)
                nc.scalar.activation(
                    ksr[h][s][:, hb:hb + HD], ones_f32, AF.Copy,
                    scale=kv_acc[h][:, s, HD:HD + 1])

        # ---------------- pass B ----------------
        # same 3-stage pipeline; qproj units trickle between attn/y units.
        with ExitStack() as bctx:
            wopool = bctx.enter_context(tc.tile_pool(name="wopool", bufs=1))
            wo_sb = [wopool.tile([P, D], BF16, tag=f"wo{s}", name=f"wo{s}") for s in range(4)]
            for s in range(4):
                nc.sync.dma_start(out=wo_sb[s], in_=wout[s * P:(s + 1) * P, :])

            attpool = bctx.enter_context(tc.tile_pool(name="attpool", bufs=3))
            zpool = bctx.enter_context(tc.tile_pool(name="zpool", bufs=2))
            ypool = bctx.enter_context(tc.tile_pool(name="ypool", bufs=2))
            qpps = bctx.enter_context(tc.tile_pool(name="qpps", bufs=2, space="PSUM"))
            atps = bctx.enter_context(tc.tile_pool(name="atps", bufs=2, space="PSUM"))
            dnps = bctx.enter_context(tc.tile_pool(name="dnps", bufs=2, space="PSUM"))
            yps = bctx.enter_context(tc.tile_pool(name="yps", bufs=2, space="PSUM"))

            atts = {}  # (g, hp) -> att_sb tile fp16

            def unit_attn(g, hp):
                aps = atps.tile([P, GS], FP32, tag="at", name="at")
                dps = dnps.tile([P, GS], FP32, tag="dn", name="dn")
                for h in (2 * hp, 2 * hp + 1):
                    first = h % 2 == 0
                    last = h % 2 == 1
                    for s in range(2):
                        qP = qPs.pop((g, h, s)) if last and s == 1 \
                            else qPs[(g, h, s)]
                        nc.tensor.matmul(
                            aps, lhsT=(kvS[h][s]), rhs=(qP),
                            start=(first and s == 0), stop=(last and s == 1),
                            skip_group_check=True)
                        nc.tensor.matmul(
                            dps, lhsT=(ksr[h][s]), rhs=(qP),
                            start=(first and s == 0), stop=(last and s == 1),
                            skip_group_check=True)
                zb = zpool.tile([P, GS], FP32, tag="zb", name="zb")
                nc.vector.reciprocal_approx_fast(zb, dps)
                att = attpool.tile([P, GS], BF16, tag=f"att{hp}",
                                   name=f"att{hp}")
                nc.vector.tensor_tensor(out=att, in0=aps, in1=zb, op=AL.mult)
                atts[(g, hp)] = att

            def unit_y(g, att, t):
                g0 = g * GS
                pso = [yps.tile([P, GS], FP32, tag="yp", name="yp")
                       for o in range(2)]
                for s in range(4):
                    for o in range(2):
                        nc.tensor.matmul(
                            pso[o], lhsT=(att[s][:, t * P:(t + 1) * P]),
                            rhs=(wo_sb[s][:, o * GS:(o + 1) * GS]),
                            start=(s == 0), stop=(s == 3))
                for o in range(2):
                    y_sb = ypool.tile([P, GS], BF16, tag=f"ysb{o}",
                                      name=f"ysb{o}")
                    nc.scalar.copy(y_sb, pso[o])
                    nc.scalar.dma_start(
                        out=y[g0 + t * P: g0 + (t + 1) * P,
                              o * GS:(o + 1) * GS],
                        in_=y_sb)

            # group order: spilled (light) groups alternate with inline
            # (heavy) ones; prepare(GORD[j+2]) = DMA readback for light,
            # qproj+elu for heavy
            GORD = [0, 1, 4, 2, 5, 3, 6, 7][:NG]

            def prepare(g):
                if g < SPILL_G:
                    readback_qP(g)
                    return []
                return [(h, s) for h in range(HPC) for s in range(2)]

            for j in range(-2, 0):    # prologue: groups at positions 0/1
                if j + 2 < len(GORD):
                    qp_units = prepare(GORD[j + 2])
                    for iq, (h, s) in enumerate(qp_units):
                        unit_qproj(GORD[j + 2], h, s, iq, qpps)

            for j in range(NG + 1):
                ga = GORD[j] if j < NG else None
                gy = GORD[j - 1] if 1 <= j <= NG else None
                qp_units = prepare(GORD[j + 2]) if j + 2 < NG else []
                att_y = [atts.pop((gy, hp)) for hp in range(4)] \
                    if gy is not None else None
                iq = 0
                for i in range(4):
                    if ga is not None:
                        unit_attn(ga, i)
                    for _ in range(4):
                        if qp_units:
                            h, s = qp_units.pop(0)
                            unit_qproj(GORD[j + 2], h, s, iq, qpps)
                            iq += 1
                    if att_y is not None:
                        unit_y(gy, att_y, i)


def build(n=SEQ):
    # Bacc (not raw Bass): its compile pipeline splits multi-waits into
    # event semaphores (TRN2 allows at most 1 sync wait per instruction).
    nc = bacc.Bacc("TRN2", target_bir_lowering=False, debug=False,
                   enable_asserts=False)
    xT = nc.declare_dram_parameter("xT", [D, n], BF16, isOutput=False)
    wq = nc.declare_dram_parameter("wq", [D, DH], BF16, isOutput=False)
    wk = nc.declare_dram_parameter("wk", [D, DH], BF16, isOutput=False)
    wv = nc.declare_dram_parameter("wv", [D, DH], BF16, isOutput=False)
    proj = nc.declare_dram_parameter("proj", [DH, F], BF16, isOutput=False)
    wout = nc.declare_dram_parameter("wout", [DH, D], BF16, isOutput=False)
    y = nc.declare_dram_parameter("y", [n, D], BF16, isOutput=True)
    qPd = nc.dram_tensor("qPd", [2 * DH * 2, n], BF16)
    with tile.TileContext(nc) as tc:
        _emit(tc, n, xT, wq, wk, wv, proj, wout, y, qPd)
    nc.finalize()
    return nc


def make_in_maps(x, w_qkv, proj_matrix, w_out):
    bf = np.float16
    x = np.asarray(x, np.float32)
    w_qkv = np.asarray(w_qkv, bf)
    proj_matrix = np.asarray(proj_matrix, bf)
    w_out = np.asarray(w_out, bf)
    in_maps = []
    for c in range(NCORES):
        b, g = c // 2, c % 2
        in_maps.append({
            "xT": np.ascontiguousarray(x[b].T.astype(bf)),
            "wq": np.ascontiguousarray(w_qkv[:, DH * g:DH * (g + 1)]),
            "wk": np.ascontiguousarray(w_qkv[:, D + DH * g:D + DH * (g + 1)]),
            "wv": np.ascontiguousarray(w_qkv[:, 2 * D + DH * g:2 * D + DH * (g + 1)]),
            "proj": np.ascontiguousarray(
                proj_matrix[HPC * g:HPC * (g + 1)].reshape(DH, F)),
            "wout": np.ascontiguousarray(w_out[DH * g:DH * (g + 1), :]),
        })
    return in_maps


_NC_CACHE = {}


def get_nc(n=SEQ):
    if n not in _NC_CACHE:
        _NC_CACHE[n] = build(n)
    return _NC_CACHE[n]


def _install_ntff_hook_shim():
    """The agent image's antenv lacks axon_hooks; recreate it so
    run_bass_kernel_spmd(trace=True) can capture NTFF profiles."""
    import sys
    import types
    try:
        from antenv.axon_hooks import get_axon_ntff_profile_hook  # noqa: F401
        return True
    except ImportError:
        pass
    try:
        from trn_agent_boot.trn_boot import _ntff_profile_via_ctypes
        import antenv
        mod = types.ModuleType("antenv.axon_hooks")
        mod._hook = _ntff_profile_via_ctypes("/opt/axon/libaxon_pjrt.so")
        mod.set_axon_ntff_profile_hook = lambda h: setattr(mod, "_hook", h)
        mod.get_axon_ntff_profile_hook = lambda: mod._hook
        sys.modules["antenv.axon_hooks"] = mod
        antenv.axon_hooks = mod
        return True
    except Exception as e:  # profiling is best-effort
        print(f"ntff hook shim failed: {e}")
        return False


def run(x, w_qkv, proj_matrix, w_out, b_out, trace=False, **kw):
    if trace:
        _install_ntff_hook_shim()
    nc = get_nc(SEQ)
    in_maps = make_in_maps(x, w_qkv, proj_matrix, w_out)
    res = run_bass_kernel_spmd(nc, in_maps, list(range(NCORES)),
                               trace=trace, **kw)
    b_out = np.asarray(b_out, np.float32)
    out = np.empty((B, SEQ, D), np.float32)
    for b in range(B):
        out[b] = np.asarray(res.results[2 * b]["y"], np.float32) \
            + np.asarray(res.results[2 * b + 1]["y"], np.float32) \
            + b_out[None, :]
    return out, res


def kernel(x, w_qkv, proj_matrix, w_out, b_out):
    out, _ = run(x, w_qkv, proj_matrix, w_out, b_out)
    return out


# revision 14
# speedup vs baseline: 1.0677x; 1.0015x over previous
"""Trainium2 Bass kernel: Performer (linear) attention + in/out projections.

Problem nn_LinearPerformerAttention_6717328851263:
  x:(4,4096,1024) f32, w_qkv:(1024,3072), proj_matrix:(16,64,256),
  w_out:(1024,1024), b_out:(1024,)

  qkv = x @ w_qkv ; split q,k,v ; per (b,h): q_proj=elu1(q@P_h), k_proj=elu1(k@P_h)
  kv = k_proj^T v ; k_sum = sum_n k_proj ; attn = (q_proj @ kv) / (q_proj@k_sum)
  out = attn @ w_out + b_out

Sharding over 8 cores: core c -> (batch b=c//2, head-group g=c%2: 8 of 16 heads).
Each core computes partial y_c = attn(b, heads_g) @ w_out[512g:512g+512, :].
Host gather: out[b] = y_(b,0) + y_(b,1) + b_out.

v1 rewrite vs baseline (594 us):
- all matmul operands bf16 (rel_fro ~3.5e-3 vs 2e-2 gate, CPU-simulated):
  f32r pays 4x cycles on <256-wide streams and the TRN2 PE p-state ramp
  (0.65/1.2/2.4 GHz; max only after 3us of CONTINUOUS execution) punishes
  any stall; bf16 is 1 cycle/row at every width.
- qT kept SBUF-resident for pass B (kills the 16 MiB DRAM spill round trip).
- kv state computed directly in [F,d] orientation (lhsT=k_projE, rhs=v|1):
  65-row streams, ~2x fewer PE cycles than the old [d,F]+transpose fixup,
  and the fixup transposes disappear (ksr is built with one tensor_scalar).
- 3-stage software pipeline in both passes: PE stream for iteration i is
  [independent GEMMs for group g | proj mms for g-1 | consumer mms for g-2]
  so matmuls never wait on the elu chain; PSUM tiles are drained to SBUF
  bf16 right after production (PSUM can only hold ~8 [128,512] tiles).
- elu1(x)=min(exp(x),1)+relu(x): exp on Scalar, relu split Scalar/Vector,
  min+add on Vector.  All elu intermediates stay f32 (DVE ops with 2-byte
  INPUTS hit a ~10x slow path; f32-in/bf16-out runs at full rate), bf16 is
  written only at the matmul-input boundary.  GpSimd runs nothing bulky
  (Q7 tensor routines are ~12x slower than DVE).
"""

import numpy as np
from contextlib import ExitStack

import ml_dtypes

import concourse.bass as bass
import concourse.bacc as bacc
import concourse.tile as tile
from concourse import mybir
from concourse.bass_utils import run_bass_kernel_spmd

FP32 = mybir.dt.float32
BF16 = mybir.dt.float16  # fp16: DVE-native 16-bit (bf16 inputs hit a slow DVE path)
AL = mybir.AluOpType
AF = mybir.ActivationFunctionType

B, SEQ, D = 4, 4096, 1024
H, HD, F = 16, 64, 256
HPC = 8            # heads per core
DH = HPC * HD      # 512 head-space dims per core
P = 128
NCORES = 8
GS = 512           # tokens per group
TPG = 4            # 128-token tiles per group


def _emit(tc, n, xT, wq, wk, wv, proj, wout, y, qPd):
    nc = tc.nc
    NG = n // GS
    SPILL_G = 4       # groups whose qproj+elu runs in pass A (qP via DRAM)

    def copy_op(idx):
        # alternate PSUM->SBUF eviction between Scalar (activation Copy)
        # and Vector engines
        return nc.scalar.copy if idx % 2 == 0 else nc.vector.tensor_copy

    ctx = ExitStack()
    with ctx:
        const = ctx.enter_context(tc.tile_pool(name="const", bufs=1))

        ones_bf = const.tile([P, P], BF16, tag="ones", name="ones")
        nc.vector.memset(ones_bf, 1.0)
        ones_f32 = const.tile([P, HD], FP32, tag="onesf", name="onesf")
        nc.vector.memset(ones_f32, 1.0)

        # proj, pair-packed [128, 256]: head 2i at partitions 0:64, head
        # 2i+1 at 64:128 (lhsT/rhs partition bases always match).
        proj_pair = [const.tile([P, F], BF16, tag=f"projp{i}", name=f"projp{i}")
                     for i in range(4)]
        for i in range(4):
            nc.sync.dma_start(out=proj_pair[i], in_=proj[i * P:(i + 1) * P, :])

        # attn lhsT, zero-padded so a head pair accumulates into one
        # [128,512] PSUM tile: kvS[h][s] [128 F-slab, 128]; cols (h%2)*64..
        # hold kv_h, other 64 cols zero.  ksr[h][s]: same but columns
        # replicate k_sum_h (denominator lands on matching partitions).
        kvS = [[const.tile([P, P], BF16, tag=f"kvS{h}_{s}", name=f"kvS{h}_{s}")
                for s in range(2)] for h in range(HPC)]
        ksr = [[const.tile([P, P], BF16, tag=f"ksr{h}_{s}", name=f"ksr{h}_{s}")
                for s in range(2)] for h in range(HPC)]
        for h in range(HPC):
            for s in range(2):
                nc.gpsimd.memset(kvS[h][s], 0.0)
                nc.gpsimd.memset(ksr[h][s], 0.0)

        # kv accumulator per head: [128 F-sub, 2 s-slabs, 65] f32.
        # col 64 = k_sum (ones column of vone).
        kv_acc = [const.tile([P, 2, HD + 1], FP32, tag=f"kva{h}", name=f"kva{h}")
                  for h in range(HPC)]

        # full-sequence qT, pair-packed [128, 4 pairs, n] fp16 (4 MiB)
        qT_sb = const.tile([P, 4, n], BF16, tag="qTs", name="qTs")

        # pass-B qproj elu pools live at ctx level: the last two pass-A
        # iterations (PE nearly idle) already emit qproj for groups 0/1
        qelupool = ctx.enter_context(tc.tile_pool(name="qelupool", bufs=4))
        qppool = ctx.enter_context(tc.tile_pool(name="qppool", bufs=3))
        qPs = {}   # (g, h, s) -> qP tile fp16

        def unit_qproj(g, h, s, idx, psum_pool, ptag="qp", spill=False):
            g0 = g * GS
            hp, hb = h // 2, (h % 2) * HD
            qp = psum_pool.tile([P, GS], FP32, tag=ptag, name=ptag)
            nc.tensor.matmul(
                qp, lhsT=(proj_pair[hp][hb:hb + HD, s * P:(s + 1) * P]),
                rhs=(qT_sb[hb:hb + HD, hp, g0:g0 + GS]),
                start=True, stop=True)
            qE = qelupool.tile([P, GS], BF16, tag="qE", name="qE")
            qR = qelupool.tile([P, GS], BF16, tag="qR", name="qR")
            nc.scalar.activation(qE, qp, AF.Exp)
            if idx % 2 == 0:
                nc.scalar.activation(qR, qp, AF.Relu)
            else:
                nc.vector.tensor_scalar_max(qR, qp, 0.0)
            qP = qppool.tile([P, GS], BF16, tag=f"qP{h}_{s}",
                             name=f"qP{h}_{s}")
            nc.vector.scalar_tensor_tensor(
                qP, in0=qE, scalar=1.0, in1=qR, op0=AL.min, op1=AL.add)
            if spill:
                r0 = (2 * h + s) * P
                nc.sync.dma_start(out=qPd[r0:r0 + P, g0:g0 + GS], in_=qP)
            else:
                qPs[(g, h, s)] = qP

        def readback_qP(g):
            g0 = g * GS
            for h in range(HPC):
                for s in range(2):
                    qP = qppool.tile([P, GS], BF16, tag=f"qP{h}_{s}",
                                     name=f"qP{h}_{s}")
                    r0 = (2 * h + s) * P
                    nc.sync.dma_start(out=qP, in_=qPd[r0:r0 + P, g0:g0 + GS])
                    qPs[(g, h, s)] = qP

        # ---------------- pass A ----------------
        # 3-stage software pipeline; within an iteration the PE stream
        # interleaves group g's qkv GEMM units with group g-1's kproj units
        # (so elu1 chases a slow trickle of PSUM tiles instead of a burst)
        # and ends with group g-2's kv matmuls (whose kP inputs got a full
        # iteration of elu latency).
        with ExitStack() as actx:
            wpool = actx.enter_context(tc.tile_pool(name="wpool", bufs=1))
            wq_sb = [wpool.tile([P, DH], BF16, tag=f"wq{s}", name=f"wq{s}") for s in range(8)]
            wk_sb = [wpool.tile([P, DH], BF16, tag=f"wk{s}", name=f"wk{s}") for s in range(8)]
            wv_sb = [wpool.tile([P, DH], BF16, tag=f"wv{s}", name=f"wv{s}") for s in range(8)]

            xtpool = actx.enter_context(tc.tile_pool(name="xtpool", bufs=2))
            ktpool = actx.enter_context(tc.tile_pool(name="ktpool", bufs=2))
            vpool = actx.enter_context(tc.tile_pool(name="vpool", bufs=3))
            elupool = actx.enter_context(tc.tile_pool(name="elupool", bufs=6))
            kppool = actx.enter_context(tc.tile_pool(name="kppool", bufs=2))
            mmps = actx.enter_context(tc.tile_pool(name="mmps", bufs=2, space="PSUM"))
            cpps = actx.enter_context(tc.tile_pool(name="cpps", bufs=4, space="PSUM"))
            kvps = actx.enter_context(tc.tile_pool(name="kvps", bufs=2, space="PSUM"))

            xT_v = xT.rearrange("(s p) m -> p s m", p=P)

            kts = {}    # g -> kt tile [128, 4, 512]
            vones = {}  # g -> vone tile [128, 4, 8, 65]
            kPs = {}    # (g, hp, tp, h) -> kP tile [128, 512] fp16
            xts = {}    # g -> xt tile [128, 8, 512]

            def fetch_xt(g):
                # per-slab sub-DMAs across the three DMA queues: the first
                # matmul only has to wait for slab 0, and queues run parallel
                g0 = g * GS
                xt = xtpool.tile([P, 8, GS], BF16, tag="xt", name="xt")
                qs = [nc.sync, nc.scalar, nc.gpsimd]
                for s in range(8):
                    qs[s % 3].dma_start(out=xt[:, s, :],
                                        in_=xT_v[:, s, g0:g0 + GS])
                xts[g] = xt

            def unit_xt(g):
                xt = xts.pop(g)
                kts[g] = ktpool.tile([P, 4, GS], BF16, tag="kt", name="kt")
                vone = vpool.tile([P, TPG, HPC, HD + 1], BF16, tag="vone",
                                  name="vone")
                nc.vector.tensor_copy(
                    vone[:, :, :, HD],
                    ones_bf[:, 0:TPG * HPC].rearrange("p (t h) -> p t h", t=TPG))
                vones[g] = vone
                return xt

            def unit_q(g, xt, fs):
                g0 = g * GS
                ps = mmps.tile([P, GS], FP32, tag="mm", name="mm")
                for s in range(8):
                    nc.tensor.matmul(
                        ps, lhsT=(wq_sb[s][:, fs * P:(fs + 1) * P]),
                        rhs=(xt[:, s, :]), start=(s == 0), stop=(s == 7))
                copy_op(fs)(qT_sb[:, fs, g0:g0 + GS], ps)

            def unit_k(g, xt, fs):
                ps = mmps.tile([P, GS], FP32, tag="mm", name="mm")
                for s in range(8):
                    nc.tensor.matmul(
                        ps, lhsT=(wk_sb[s][:, fs * P:(fs + 1) * P]),
                        rhs=(xt[:, s, :]), start=(s == 0), stop=(s == 7))
                copy_op(fs + 1)(kts[g][:, fs, :], ps)

            def unit_v(g, xt, t):
                ps = mmps.tile([P, GS], FP32, tag="mm", name="mm")
                for s in range(8):
                    nc.tensor.matmul(
                        ps, lhsT=(xt[:, s, t * P:(t + 1) * P]),
                        rhs=(wv_sb[s]), start=(s == 0), stop=(s == 7))
                copy_op(t)(
                    vones[g][:, t, :, 0:HD],
                    ps.rearrange("p (h e) -> p h e", h=HPC))

            def unit_kproj(g, hp, tp, h, idx):
                # c[tokens, 2ti x 256F] = k_h @ P_h, then elu1 -> kP bf16.
                # Even/odd heads sit at base partitions 0/64 so the PE runs
                # them in disjoint row groups.
                hb = (h % 2) * HD
                kt = kts[g]
                c = cpps.tile([P, GS], FP32, tag="kc", name="kc")
                for ti in range(2):
                    t = tp * 2 + ti
                    nc.tensor.matmul(
                        c[:, ti * F:(ti + 1) * F],
                        lhsT=(kt[hb:hb + HD, hp, t * P:(t + 1) * P]),
                        rhs=(proj_pair[hp][hb:hb + HD, :]),
                        start=True, stop=True)
                kE = elupool.tile([P, GS], BF16, tag="kE", name="kE")
                kR = elupool.tile([P, GS], BF16, tag="kR", name="kR")
                nc.scalar.activation(kE, c, AF.Exp)
                if idx % 16 < 10:  # balance Scalar vs Vector load
                    nc.scalar.activation(kR, c, AF.Relu)
                else:
                    nc.vector.tensor_scalar_max(kR, c, 0.0)
                kP = kppool.tile([P, GS], BF16, tag=f"kP{hp}_{tp}_{h % 2}",
                                 name=f"kP{hp}_{tp}_{h % 2}")
                nc.vector.scalar_tensor_tensor(
                    kP, in0=kE, scalar=1.0, in1=kR, op0=AL.min, op1=AL.add)
                kPs[(g, hp, tp, h)] = kP

            def unit_kv(g, hp, h):
                # kv[f, d] += k_projE^T [v|1] per (head, F-slab), PSUM
                # accumulated over the 4 token tiles, folded into kv_acc f32.
                vone = vones[g]
                kv_ps = kvps.tile([P, 2, P], FP32, tag="kv", name="kv")
                for s in range(2):
                    for t in range(TPG):
                        tp, ti = t // 2, t % 2
                        kP = kPs.pop((g, hp, tp, h)) if s == 1 and t == TPG - 1 \
                            else kPs[(g, hp, tp, h)]
                        nc.tensor.matmul(
                            kv_ps[:, s, 0:HD + 1],
                            lhsT=(kP[:, ti * F + s * P: ti * F + s * P + P]),
                            rhs=(vone[:, t, h, :]),
                            start=(t == 0), stop=(t == TPG - 1),
                            skip_group_check=True)
                if g == 0:
                    nc.vector.tensor_copy(kv_acc[h], kv_ps[:, :, 0:HD + 1])
                else:
                    nc.vector.tensor_tensor(
                        out=kv_acc[h], in0=kv_ps[:, :, 0:HD + 1],
                        in1=kv_acc[h], op=AL.add)

            fetch_xt(0)  # x tile 0 in flight before the weight loads
            for s in range(8):
                nc.scalar.dma_start(out=wq_sb[s], in_=wq[s * P:(s + 1) * P, :])
                nc.gpsimd.dma_start(out=wk_sb[s], in_=wk[s * P:(s + 1) * P, :])
                (nc.scalar if s % 2 else nc.gpsimd).dma_start(
                    out=wv_sb[s], in_=wv[s * P:(s + 1) * P, :])

            for it in range(NG + 2):
                g = it            # group doing qkv GEMMs
                gk = it - 1       # group doing kproj+elu
                gv = it - 2       # group doing kv accumulation
                xt = unit_xt(g) if g < NG else None
                # proj-side units for this iteration: 16 kproj units for
                # group gk, plus (mid iterations) 16 spill-qproj units for
                # group it-2 -- both share the kc psum rotation and get
                # interleaved among the 12 qkv GEMM units
                mid_units = []
                if 0 <= gk < NG:
                    mid_units += [("kp", hp, tp, h)
                                  for hp in range(HPC // 2)
                                  for tp in range(2)
                                  for h in (2 * hp, 2 * hp + 1)]
                if 2 <= it < 2 + 2 * SPILL_G:
                    gq, half = (it - 2) // 2, (it - 2) % 2
                    allu = [(h, s) for h in range(HPC) for s in range(2)]
                    qsp = [("qp", gq, h, s)
                           for (h, s) in allu[half * 8:half * 8 + 8]]
                    mix = []
                    while mid_units or qsp:
                        if mid_units:
                            mix.append(mid_units.pop(0))
                        if qsp:
                            mix.append(qsp.pop(0))
                    mid_units = mix
                if 0 <= gv < NG:
                    mid_units += [("kv", hp, h)
                                  for hp in range(HPC // 2)
                                  for h in (2 * hp, 2 * hp + 1)]
                n_mid = len(mid_units)
                qkv_units = []
                if g < NG:
                    qkv_units = [("q", fs) for fs in range(4)] + \
                                [("k", fs) for fs in range(4)] + \
                                [("v", t) for t in range(4)]
                ik = 0
                iqp = 0

                def pop_mid():
                    nonlocal ik, iqp
                    u = mid_units.pop(0)
                    if u[0] == "kp":
                        _, hp, tp, h = u
                        unit_kproj(gk, hp, tp, h, ik)
                        ik += 1
                    elif u[0] == "qp":
                        _, gq, h, s = u
                        unit_qproj(gq, h, s, iqp, cpps, ptag="kc",
                                   spill=True)
                        iqp += 1
                    else:
                        _, hp, h = u
                        unit_kv(gv, hp, h)

                for iu, u in enumerate(qkv_units):
                    kind, a = u
                    if kind == "q":
                        unit_q(g, xt, a)
                    elif kind == "k":
                        unit_k(g, xt, a)
                    else:
                        unit_v(g, xt, a)
                    if iu == 3 and g + 1 < NG:
                        fetch_xt(g + 1)
                    while len(mid_units) * 12 > (11 - iu) * n_mid \
                            and mid_units:
                        pop_mid()
                while mid_units:
                    pop_mid()

        # ---------------- kv fixup: kv_acc -> kvS/ksr (bf16) ----------------
        for h in range(HPC):
            hb = (h % 2) * HD
            for s in range(2):
                nc.vector.tensor_copy(kvS[h][s][:, hb:hb + HD],
                                      kv_acc[h][:, s, 0:HD])
                # ksr cols hb:hb+64 = k_sum[f] broadcast along free dim
                # (scalar engine: copy of ones scaled per-partition by k_sum)
                nc.scalar.activation(
                    ksr[h][s][:, hb:hb + HD], ones_f32, AF.Copy,
                    scale=kv_acc[h][:, s, HD:HD + 1])

        # ---------------- pass B ----------------
        # same 3-stage pipeline; qproj units trickle between attn/y units.
        with ExitStack() as bctx:
            wopool = bctx.enter_context(tc.tile_pool(name="wopool", bufs=1))
            wo_sb = [wopool.tile([P, D], BF16, tag=f"wo{s}", name=f"wo{s}") for s in range(4)]
            for s in range(4):
                nc.sync.dma_start(out=wo_sb[s], in_=wout[s * P:(s + 1) * P, :])

            attpool = bctx.enter_context(tc.tile_pool(name="attpool", bufs=3))
            zpool = bctx.enter_context(tc.tile_pool(name="zpool", bufs=2))
            ypool = bctx.enter_context(tc.tile_pool(name="ypool", bufs=2))
            qpps = bctx.enter_context(tc.tile_pool(name="qpps", bufs=3, space="PSUM"))
            atps = bctx.enter_context(tc.tile_pool(name="atps", bufs=2, space="PSUM"))
            dnps = bctx.enter_context(tc.tile_pool(name="dnps", bufs=1, space="PSUM"))
            yps = bctx.enter_context(tc.tile_pool(name="yps", bufs=2, space="PSUM"))

            atts = {}  # (g, hp) -> att_sb tile fp16

            def unit_attn(g, hp):
                aps = atps.tile([P, GS], FP32, tag="at", name="at")
                dps = dnps.tile([P, GS], FP32, tag="dn", name="dn")
                for h in (2 * hp, 2 * hp + 1):
                    first = h % 2 == 0
                    last = h % 2 == 1
                    for s in range(2):
                        qP = qPs.pop((g, h, s)) if last and s == 1 \
                            else qPs[(g, h, s)]
                        nc.tensor.matmul(
                            aps, lhsT=(kvS[h][s]), rhs=(qP),
                            start=(first and s == 0), stop=(last and s == 1),
                            skip_group_check=True)
                        nc.tensor.matmul(
                            dps, lhsT=(ksr[h][s]), rhs=(qP),
                            start=(first and s == 0), stop=(last and s == 1),
                            skip_group_check=True)
                zb = zpool.tile([P, GS], FP32, tag="zb", name="zb")
                nc.vector.reciprocal_approx_fast(zb, dps)
                att = attpool.tile([P, GS], BF16, tag=f"att{hp}",
                                   name=f"att{hp}")
                nc.vector.tensor_tensor(out=att, in0=aps, in1=zb, op=AL.mult)
                atts[(g, hp)] = att

            def unit_y(g, att, t):
                g0 = g * GS
                pso = [yps.tile([P, GS], FP32, tag="yp", name="yp")
                       for o in range(2)]
                for s in range(4):
                    for o in range(2):
                        nc.tensor.matmul(
                            pso[o], lhsT=(att[s][:, t * P:(t + 1) * P]),
                            rhs=(wo_sb[s][:, o * GS:(o + 1) * GS]),
                            start=(s == 0), stop=(s == 3))
                for o in range(2):
                    y_sb = ypool.tile([P, GS], BF16, tag=f"ysb{o}",
                                      name=f"ysb{o}")
                    nc.scalar.copy(y_sb, pso[o])
                    nc.scalar.dma_start(
                        out=y[g0 + t * P: g0 + (t + 1) * P,
                              o * GS:(o + 1) * GS],
                        in_=y_sb)

            # group order: spilled (light) groups alternate with inline
            # (heavy) ones; prepare(GORD[j+2]) = DMA readback for light,
            # qproj+elu for heavy
            GORD = [0, 1, 4, 2, 5, 3, 6, 7][:NG]

            def prepare(g):
                if g < SPILL_G:
                    readback_qP(g)
                    return []
                return [(h, s) for h in range(HPC) for s in range(2)]

            for j in range(-2, 0):    # prologue: groups at positions 0/1
                if j + 2 < len(GORD):
                    qp_units = prepare(GORD[j + 2])
                    for iq, (h, s) in enumerate(qp_units):
                        unit_qproj(GORD[j + 2], h, s, iq, qpps)

            for j in range(NG + 1):
                ga = GORD[j] if j < NG else None
                gy = GORD[j - 1] if 1 <= j <= NG else None
                qp_units = prepare(GORD[j + 2]) if j + 2 < NG else []
                att_y = [atts.pop((gy, hp)) for hp in range(4)] \
                    if gy is not None else None
                iq = 0
                for i in range(4):
                    if ga is not None:
                        unit_attn(ga, i)
                    for _ in range(4):
                        if qp_units:
                            h, s = qp_units.pop(0)
                            unit_qproj(GORD[j + 2], h, s, iq, qpps)
                            iq += 1
                    if att_y is not None:
                        unit_y(gy, att_y, i)


def build(n=SEQ):
    # Bacc (not raw Bass): its compile pipeline splits multi-waits into
    # event semaphores (TRN2 allows at most 1 sync wait per instruction).
    nc = bacc.Bacc("TRN2", target_bir_lowering=False, debug=False,
                   enable_asserts=False)
    xT = nc.declare_dram_parameter("xT", [D, n], BF16, isOutput=False)
    wq = nc.declare_dram_parameter("wq", [D, DH], BF16, isOutput=False)
    wk = nc.declare_dram_parameter("wk", [D, DH], BF16, isOutput=False)
    wv = nc.declare_dram_parameter("wv", [D, DH], BF16, isOutput=False)
    proj = nc.declare_dram_parameter("proj", [DH, F], BF16, isOutput=False)
    wout = nc.declare_dram_parameter("wout", [DH, D], BF16, isOutput=False)
    y = nc.declare_dram_parameter("y", [n, D], BF16, isOutput=True)
    qPd = nc.dram_tensor("qPd", [2 * DH * 2, n], BF16)
    with tile.TileContext(nc) as tc:
        _emit(tc, n, xT, wq, wk, wv, proj, wout, y, qPd)
    nc.finalize()
    return nc


def make_in_maps(x, w_qkv, proj_matrix, w_out):
    bf = np.float16
    x = np.asarray(x, np.float32)
    w_qkv = np.asarray(w_qkv, bf)
    proj_matrix = np.asarray(proj_matrix, bf)
    w_out = np.asarray(w_out, bf)
    in_maps = []
    for c in range(NCORES):
        b, g = c // 2, c % 2
        in_maps.append({
            "xT": np.ascontiguousarray(x[b].T.astype(bf)),
            "wq": np.ascontiguousarray(w_qkv[:, DH * g:DH * (g + 1)]),
            "wk": np.ascontiguousarray(w_qkv[:, D + DH * g:D + DH * (g + 1)]),
            "wv": np.ascontiguousarray(w_qkv[:, 2 * D + DH * g:2 * D + DH * (g + 1)]),
            "proj": np.ascontiguousarray(
                proj_matrix[HPC * g:HPC * (g + 1)].reshape(DH, F)),
            "wout": np.ascontiguousarray(w_out[DH * g:DH * (g + 1), :]),
        })
    return in_maps


_NC_CACHE = {}


def get_nc(n=SEQ):
    if n not in _NC_CACHE:
        _NC_CACHE[n] = build(n)
    return _NC_CACHE[n]


def _install_ntff_hook_shim():
    """The agent image's antenv lacks axon_hooks; recreate it so
    run_bass_kernel_spmd(trace=True) can capture NTFF profiles."""
    import sys
    import types
    try:
        from antenv.axon_hooks import get_axon_ntff_profile_hook  # noqa: F401
        return True
    except ImportError:
        pass
    try:
        from trn_agent_boot.trn_boot import _ntff_profile_via_ctypes
        import antenv
        mod = types.ModuleType("antenv.axon_hooks")
        mod._hook = _ntff_profile_via_ctypes("/opt/axon/libaxon_pjrt.so")
        mod.set_axon_ntff_profile_hook = lambda h: setattr(mod, "_hook", h)
        mod.get_axon_ntff_profile_hook = lambda: mod._hook
        sys.modules["antenv.axon_hooks"] = mod
        antenv.axon_hooks = mod
        return True
    except Exception as e:  # profiling is best-effort
        print(f"ntff hook shim failed: {e}")
        return False


def run(x, w_qkv, proj_matrix, w_out, b_out, trace=False, **kw):
    if trace:
        _install_ntff_hook_shim()
    nc = get_nc(SEQ)
    in_maps = make_in_maps(x, w_qkv, proj_matrix, w_out)
    res = run_bass_kernel_spmd(nc, in_maps, list(range(NCORES)),
                               trace=trace, **kw)
    b_out = np.asarray(b_out, np.float32)
    out = np.empty((B, SEQ, D), np.float32)
    for b in range(B):
        out[b] = np.asarray(res.results[2 * b]["y"], np.float32) \
            + np.asarray(res.results[2 * b + 1]["y"], np.float32) \
            + b_out[None, :]
    return out, res


def kernel(x, w_qkv, proj_matrix, w_out, b_out):
    out, _ = run(x, w_qkv, proj_matrix, w_out, b_out)
    return out


# revision 15
# speedup vs baseline: 1.0692x; 1.0014x over previous
"""Trainium2 Bass kernel: Performer (linear) attention + in/out projections.

Problem nn_LinearPerformerAttention_6717328851263:
  x:(4,4096,1024) f32, w_qkv:(1024,3072), proj_matrix:(16,64,256),
  w_out:(1024,1024), b_out:(1024,)

  qkv = x @ w_qkv ; split q,k,v ; per (b,h): q_proj=elu1(q@P_h), k_proj=elu1(k@P_h)
  kv = k_proj^T v ; k_sum = sum_n k_proj ; attn = (q_proj @ kv) / (q_proj@k_sum)
  out = attn @ w_out + b_out

Sharding over 8 cores: core c -> (batch b=c//2, head-group g=c%2: 8 of 16 heads).
Each core computes partial y_c = attn(b, heads_g) @ w_out[512g:512g+512, :].
Host gather: out[b] = y_(b,0) + y_(b,1) + b_out.

v1 rewrite vs baseline (594 us):
- all matmul operands bf16 (rel_fro ~3.5e-3 vs 2e-2 gate, CPU-simulated):
  f32r pays 4x cycles on <256-wide streams and the TRN2 PE p-state ramp
  (0.65/1.2/2.4 GHz; max only after 3us of CONTINUOUS execution) punishes
  any stall; bf16 is 1 cycle/row at every width.
- qT kept SBUF-resident for pass B (kills the 16 MiB DRAM spill round trip).
- kv state computed directly in [F,d] orientation (lhsT=k_projE, rhs=v|1):
  65-row streams, ~2x fewer PE cycles than the old [d,F]+transpose fixup,
  and the fixup transposes disappear (ksr is built with one tensor_scalar).
- 3-stage software pipeline in both passes: PE stream for iteration i is
  [independent GEMMs for group g | proj mms for g-1 | consumer mms for g-2]
  so matmuls never wait on the elu chain; PSUM tiles are drained to SBUF
  bf16 right after production (PSUM can only hold ~8 [128,512] tiles).
- elu1(x)=min(exp(x),1)+relu(x): exp on Scalar, relu split Scalar/Vector,
  min+add on Vector.  All elu intermediates stay f32 (DVE ops with 2-byte
  INPUTS hit a ~10x slow path; f32-in/bf16-out runs at full rate), bf16 is
  written only at the matmul-input boundary.  GpSimd runs nothing bulky
  (Q7 tensor routines are ~12x slower than DVE).
"""

import numpy as np
from contextlib import ExitStack

import ml_dtypes

import concourse.bass as bass
import concourse.bacc as bacc
import concourse.tile as tile
from concourse import mybir
from concourse.bass_utils import run_bass_kernel_spmd

FP32 = mybir.dt.float32
BF16 = mybir.dt.float16  # fp16: DVE-native 16-bit (bf16 inputs hit a slow DVE path)
AL = mybir.AluOpType
AF = mybir.ActivationFunctionType

B, SEQ, D = 4, 4096, 1024
H, HD, F = 16, 64, 256
HPC = 8            # heads per core
DH = HPC * HD      # 512 head-space dims per core
P = 128
NCORES = 8
GS = 512           # tokens per group
TPG = 4            # 128-token tiles per group


def _emit(tc, n, xT, wq, wk, wv, proj, wout, y, qPd):
    nc = tc.nc
    NG = n // GS
    SPILL_G = 4       # groups whose qproj+elu runs in pass A (qP via DRAM)

    def copy_op(idx):
        # alternate PSUM->SBUF eviction between Scalar (activation Copy)
        # and Vector engines
        return nc.scalar.copy if idx % 2 == 0 else nc.vector.tensor_copy

    ctx = ExitStack()
    with ctx:
        const = ctx.enter_context(tc.tile_pool(name="const", bufs=1))

        ones_bf = const.tile([P, P], BF16, tag="ones", name="ones")
        nc.vector.memset(ones_bf, 1.0)
        ones_f32 = const.tile([P, HD], FP32, tag="onesf", name="onesf")
        nc.vector.memset(ones_f32, 1.0)

        # proj, pair-packed [128, 256]: head 2i at partitions 0:64, head
        # 2i+1 at 64:128 (lhsT/rhs partition bases always match).
        proj_pair = [const.tile([P, F], BF16, tag=f"projp{i}", name=f"projp{i}")
                     for i in range(4)]
        for i in range(4):
            nc.sync.dma_start(out=proj_pair[i], in_=proj[i * P:(i + 1) * P, :])

        # attn lhsT, zero-padded so a head pair accumulates into one
        # [128,512] PSUM tile: kvS[h][s] [128 F-slab, 128]; cols (h%2)*64..
        # hold kv_h, other 64 cols zero.  ksr[h][s]: same but columns
        # replicate k_sum_h (denominator lands on matching partitions).
        kvS = [[const.tile([P, P], BF16, tag=f"kvS{h}_{s}", name=f"kvS{h}_{s}")
                for s in range(2)] for h in range(HPC)]
        ksr = [[const.tile([P, P], BF16, tag=f"ksr{h}_{s}", name=f"ksr{h}_{s}")
                for s in range(2)] for h in range(HPC)]
        for h in range(HPC):
            for s in range(2):
                nc.gpsimd.memset(kvS[h][s], 0.0)
                nc.gpsimd.memset(ksr[h][s], 0.0)

        # kv accumulator per head: [128 F-sub, 2 s-slabs, 65] f32.
        # col 64 = k_sum (ones column of vone).
        kv_acc = [const.tile([P, 2, HD + 1], FP32, tag=f"kva{h}", name=f"kva{h}")
                  for h in range(HPC)]

        # full-sequence qT, pair-packed [128, 4 pairs, n] fp16 (4 MiB)
        qT_sb = const.tile([P, 4, n], BF16, tag="qTs", name="qTs")

        # pass-B qproj elu pools live at ctx level: the last two pass-A
        # iterations (PE nearly idle) already emit qproj for groups 0/1
        qelupool = ctx.enter_context(tc.tile_pool(name="qelupool", bufs=4))
        qppool = ctx.enter_context(tc.tile_pool(name="qppool", bufs=3))
        qPs = {}   # (g, h, s) -> qP tile fp16

        def unit_qproj(g, h, s, idx, psum_pool, ptag="qp", spill=False):
            g0 = g * GS
            hp, hb = h // 2, (h % 2) * HD
            qp = psum_pool.tile([P, GS], FP32, tag=ptag, name=ptag)
            nc.tensor.matmul(
                qp, lhsT=(proj_pair[hp][hb:hb + HD, s * P:(s + 1) * P]),
                rhs=(qT_sb[hb:hb + HD, hp, g0:g0 + GS]),
                start=True, stop=True)
            qE = qelupool.tile([P, GS], BF16, tag="qE", name="qE")
            qR = qelupool.tile([P, GS], BF16, tag="qR", name="qR")
            nc.scalar.activation(qE, qp, AF.Exp)
            if idx % 2 == 0:
                nc.scalar.activation(qR, qp, AF.Relu)
            else:
                nc.vector.tensor_scalar_max(qR, qp, 0.0)
            qP = qppool.tile([P, GS], BF16, tag=f"qP{h}_{s}",
                             name=f"qP{h}_{s}")
            nc.vector.scalar_tensor_tensor(
                qP, in0=qE, scalar=1.0, in1=qR, op0=AL.min, op1=AL.add)
            if spill:
                r0 = (2 * h + s) * P
                (nc.sync if idx % 2 else nc.gpsimd).dma_start(
                    out=qPd[r0:r0 + P, g0:g0 + GS], in_=qP)
            else:
                qPs[(g, h, s)] = qP

        def readback_qP(g):
            g0 = g * GS
            qs = [nc.sync, nc.scalar, nc.gpsimd]
            for i, (h, s) in enumerate([(h, s) for h in range(HPC)
                                        for s in range(2)]):
                qP = qppool.tile([P, GS], BF16, tag=f"qP{h}_{s}",
                                 name=f"qP{h}_{s}")
                r0 = (2 * h + s) * P
                qs[i % 3].dma_start(out=qP, in_=qPd[r0:r0 + P, g0:g0 + GS])
                qPs[(g, h, s)] = qP

        # ---------------- pass A ----------------
        # 3-stage software pipeline; within an iteration the PE stream
        # interleaves group g's qkv GEMM units with group g-1's kproj units
        # (so elu1 chases a slow trickle of PSUM tiles instead of a burst)
        # and ends with group g-2's kv matmuls (whose kP inputs got a full
        # iteration of elu latency).
        with ExitStack() as actx:
            wpool = actx.enter_context(tc.tile_pool(name="wpool", bufs=1))
            wq_sb = [wpool.tile([P, DH], BF16, tag=f"wq{s}", name=f"wq{s}") for s in range(8)]
            wk_sb = [wpool.tile([P, DH], BF16, tag=f"wk{s}", name=f"wk{s}") for s in range(8)]
            wv_sb = [wpool.tile([P, DH], BF16, tag=f"wv{s}", name=f"wv{s}") for s in range(8)]

            xtpool = actx.enter_context(tc.tile_pool(name="xtpool", bufs=2))
            ktpool = actx.enter_context(tc.tile_pool(name="ktpool", bufs=2))
            vpool = actx.enter_context(tc.tile_pool(name="vpool", bufs=3))
            elupool = actx.enter_context(tc.tile_pool(name="elupool", bufs=6))
            kppool = actx.enter_context(tc.tile_pool(name="kppool", bufs=2))
            mmps = actx.enter_context(tc.tile_pool(name="mmps", bufs=2, space="PSUM"))
            cpps = actx.enter_context(tc.tile_pool(name="cpps", bufs=4, space="PSUM"))
            kvps = actx.enter_context(tc.tile_pool(name="kvps", bufs=2, space="PSUM"))

            xT_v = xT.rearrange("(s p) m -> p s m", p=P)

            kts = {}    # g -> kt tile [128, 4, 512]
            vones = {}  # g -> vone tile [128, 4, 8, 65]
            kPs = {}    # (g, hp, tp, h) -> kP tile [128, 512] fp16
            xts = {}    # g -> xt tile [128, 8, 512]

            def fetch_xt(g):
                # per-slab sub-DMAs across the three DMA queues: the first
                # matmul only has to wait for slab 0, and queues run parallel
                g0 = g * GS
                xt = xtpool.tile([P, 8, GS], BF16, tag="xt", name="xt")
                qs = [nc.sync, nc.scalar, nc.gpsimd]
                for s in range(8):
                    qs[s % 3].dma_start(out=xt[:, s, :],
                                        in_=xT_v[:, s, g0:g0 + GS])
                xts[g] = xt

            def unit_xt(g):
                xt = xts.pop(g)
                kts[g] = ktpool.tile([P, 4, GS], BF16, tag="kt", name="kt")
                vone = vpool.tile([P, TPG, HPC, HD + 1], BF16, tag="vone",
                                  name="vone")
                nc.vector.tensor_copy(
                    vone[:, :, :, HD],
                    ones_bf[:, 0:TPG * HPC].rearrange("p (t h) -> p t h", t=TPG))
                vones[g] = vone
                return xt

            def unit_q(g, xt, fs):
                g0 = g * GS
                ps = mmps.tile([P, GS], FP32, tag="mm", name="mm")
                for s in range(8):
                    nc.tensor.matmul(
                        ps, lhsT=(wq_sb[s][:, fs * P:(fs + 1) * P]),
                        rhs=(xt[:, s, :]), start=(s == 0), stop=(s == 7))
                copy_op(fs)(qT_sb[:, fs, g0:g0 + GS], ps)

            def unit_k(g, xt, fs):
                ps = mmps.tile([P, GS], FP32, tag="mm", name="mm")
                for s in range(8):
                    nc.tensor.matmul(
                        ps, lhsT=(wk_sb[s][:, fs * P:(fs + 1) * P]),
                        rhs=(xt[:, s, :]), start=(s == 0), stop=(s == 7))
                copy_op(fs + 1)(kts[g][:, fs, :], ps)

            def unit_v(g, xt, t):
                ps = mmps.tile([P, GS], FP32, tag="mm", name="mm")
                for s in range(8):
                    nc.tensor.matmul(
                        ps, lhsT=(xt[:, s, t * P:(t + 1) * P]),
                        rhs=(wv_sb[s]), start=(s == 0), stop=(s == 7))
                copy_op(t)(
                    vones[g][:, t, :, 0:HD],
                    ps.rearrange("p (h e) -> p h e", h=HPC))

            def unit_kproj(g, hp, tp, h, idx):
                # c[tokens, 2ti x 256F] = k_h @ P_h, then elu1 -> kP bf16.
                # Even/odd heads sit at base partitions 0/64 so the PE runs
                # them in disjoint row groups.
                hb = (h % 2) * HD
                kt = kts[g]
                c = cpps.tile([P, GS], FP32, tag="kc", name="kc")
                for ti in range(2):
                    t = tp * 2 + ti
                    nc.tensor.matmul(
                        c[:, ti * F:(ti + 1) * F],
                        lhsT=(kt[hb:hb + HD, hp, t * P:(t + 1) * P]),
                        rhs=(proj_pair[hp][hb:hb + HD, :]),
                        start=True, stop=True)
                kE = elupool.tile([P, GS], BF16, tag="kE", name="kE")
                kR = elupool.tile([P, GS], BF16, tag="kR", name="kR")
                nc.scalar.activation(kE, c, AF.Exp)
                if idx % 16 < 10:  # balance Scalar vs Vector load
                    nc.scalar.activation(kR, c, AF.Relu)
                else:
                    nc.vector.tensor_scalar_max(kR, c, 0.0)
                kP = kppool.tile([P, GS], BF16, tag=f"kP{hp}_{tp}_{h % 2}",
                                 name=f"kP{hp}_{tp}_{h % 2}")
                nc.vector.scalar_tensor_tensor(
                    kP, in0=kE, scalar=1.0, in1=kR, op0=AL.min, op1=AL.add)
                kPs[(g, hp, tp, h)] = kP

            def unit_kv(g, hp, h):
                # kv[f, d] += k_projE^T [v|1] per (head, F-slab), PSUM
                # accumulated over the 4 token tiles, folded into kv_acc f32.
                vone = vones[g]
                kv_ps = kvps.tile([P, 2, P], FP32, tag="kv", name="kv")
                for s in range(2):
                    for t in range(TPG):
                        tp, ti = t // 2, t % 2
                        kP = kPs.pop((g, hp, tp, h)) if s == 1 and t == TPG - 1 \
                            else kPs[(g, hp, tp, h)]
                        nc.tensor.matmul(
                            kv_ps[:, s, 0:HD + 1],
                            lhsT=(kP[:, ti * F + s * P: ti * F + s * P + P]),
                            rhs=(vone[:, t, h, :]),
                            start=(t == 0), stop=(t == TPG - 1),
                            skip_group_check=True)
                if g == 0:
                    nc.vector.tensor_copy(kv_acc[h], kv_ps[:, :, 0:HD + 1])
                else:
                    nc.vector.tensor_tensor(
                        out=kv_acc[h], in0=kv_ps[:, :, 0:HD + 1],
                        in1=kv_acc[h], op=AL.add)

            fetch_xt(0)  # x tile 0 in flight before the weight loads
            for s in range(8):
                nc.scalar.dma_start(out=wq_sb[s], in_=wq[s * P:(s + 1) * P, :])
                nc.gpsimd.dma_start(out=wk_sb[s], in_=wk[s * P:(s + 1) * P, :])
            for s in range(8):   # v weights are needed only mid-iteration
                (nc.scalar if s % 2 else nc.gpsimd).dma_start(
                    out=wv_sb[s], in_=wv[s * P:(s + 1) * P, :])

            for it in range(NG + 2):
                g = it            # group doing qkv GEMMs
                gk = it - 1       # group doing kproj+elu
                gv = it - 2       # group doing kv accumulation
                xt = unit_xt(g) if g < NG else None
                # proj-side units for this iteration: 16 kproj units for
                # group gk, plus (mid iterations) 16 spill-qproj units for
                # group it-2 -- both share the kc psum rotation and get
                # interleaved among the 12 qkv GEMM units
                mid_units = []
                if 0 <= gk < NG:
                    mid_units += [("kp", hp, tp, h)
                                  for hp in range(HPC // 2)
                                  for tp in range(2)
                                  for h in (2 * hp, 2 * hp + 1)]
                if 2 <= it < 2 + 2 * SPILL_G:
                    gq, half = (it - 2) // 2, (it - 2) % 2
                    allu = [(h, s) for h in range(HPC) for s in range(2)]
                    qsp = [("qp", gq, h, s)
                           for (h, s) in allu[half * 8:half * 8 + 8]]
                    mix = []
                    while mid_units or qsp:
                        if mid_units:
                            mix.append(mid_units.pop(0))
                        if qsp:
                            mix.append(qsp.pop(0))
                    mid_units = mix
                if 0 <= gv < NG:
                    mid_units += [("kv", hp, h)
                                  for hp in range(HPC // 2)
                                  for h in (2 * hp, 2 * hp + 1)]
                n_mid = len(mid_units)
                qkv_units = []
                if g < NG:
                    qkv_units = [("q", fs) for fs in range(4)] + \
                                [("k", fs) for fs in range(4)] + \
                                [("v", t) for t in range(4)]
                ik = 0
                iqp = 0

                def pop_mid():
                    nonlocal ik, iqp
                    u = mid_units.pop(0)
                    if u[0] == "kp":
                        _, hp, tp, h = u
                        unit_kproj(gk, hp, tp, h, ik)
                        ik += 1
                    elif u[0] == "qp":
                        _, gq, h, s = u
                        unit_qproj(gq, h, s, iqp, cpps, ptag="kc",
                                   spill=True)
                        iqp += 1
                    else:
                        _, hp, h = u
                        unit_kv(gv, hp, h)

                for iu, u in enumerate(qkv_units):
                    kind, a = u
                    if kind == "q":
                        unit_q(g, xt, a)
                    elif kind == "k":
                        unit_k(g, xt, a)
                    else:
                        unit_v(g, xt, a)
                    if iu == 3 and g + 1 < NG:
                        fetch_xt(g + 1)
                    while len(mid_units) * 12 > (11 - iu) * n_mid \
                            and mid_units:
                        pop_mid()
                while mid_units:
                    pop_mid()
                if it == 6:
                    readback_qP(0)
                if it == 7:
                    readback_qP(1)

        # ---------------- kv fixup: kv_acc -> kvS/ksr (bf16) ----------------
        for h in range(HPC):
            hb = (h % 2) * HD
            for s in range(2):
                nc.vector.tensor_copy(kvS[h][s][:, hb:hb + HD],
                                      kv_acc[h][:, s, 0:HD])
                # ksr cols hb:hb+64 = k_sum[f] broadcast along free dim
                # (scalar engine: copy of ones scaled per-partition by k_sum)
                nc.scalar.activation(
                    ksr[h][s][:, hb:hb + HD], ones_f32, AF.Copy,
                    scale=kv_acc[h][:, s, HD:HD + 1])

        # ---------------- pass B ----------------
        # same 3-stage pipeline; qproj units trickle between attn/y units.
        with ExitStack() as bctx:
            wopool = bctx.enter_context(tc.tile_pool(name="wopool", bufs=1))
            wo_sb = [wopool.tile([P, D], BF16, tag=f"wo{s}", name=f"wo{s}") for s in range(4)]
            for s in range(4):
                nc.sync.dma_start(out=wo_sb[s], in_=wout[s * P:(s + 1) * P, :])

            attpool = bctx.enter_context(tc.tile_pool(name="attpool", bufs=3))
            zpool = bctx.enter_context(tc.tile_pool(name="zpool", bufs=2))
            ypool = bctx.enter_context(tc.tile_pool(name="ypool", bufs=2))
            qpps = bctx.enter_context(tc.tile_pool(name="qpps", bufs=3, space="PSUM"))
            atps = bctx.enter_context(tc.tile_pool(name="atps", bufs=2, space="PSUM"))
            dnps = bctx.enter_context(tc.tile_pool(name="dnps", bufs=1, space="PSUM"))
            yps = bctx.enter_context(tc.tile_pool(name="yps", bufs=2, space="PSUM"))

            atts = {}  # (g, hp) -> att_sb tile fp16

            def unit_attn(g, hp):
                aps = atps.tile([P, GS], FP32, tag="at", name="at")
                dps = dnps.tile([P, GS], FP32, tag="dn", name="dn")
                for h in (2 * hp, 2 * hp + 1):
                    first = h % 2 == 0
                    last = h % 2 == 1
                    for s in range(2):
                        qP = qPs.pop((g, h, s)) if last and s == 1 \
                            else qPs[(g, h, s)]
                        nc.tensor.matmul(
                            aps, lhsT=(kvS[h][s]), rhs=(qP),
                            start=(first and s == 0), stop=(last and s == 1),
                            skip_group_check=True)
                        nc.tensor.matmul(
                            dps, lhsT=(ksr[h][s]), rhs=(qP),
                            start=(first and s == 0), stop=(last and s == 1),
                            skip_group_check=True)
                zb = zpool.tile([P, GS], FP32, tag="zb", name="zb")
                nc.vector.reciprocal_approx_fast(zb, dps)
                att = attpool.tile([P, GS], BF16, tag=f"att{hp}",
                                   name=f"att{hp}")
                nc.vector.tensor_tensor(out=att, in0=aps, in1=zb, op=AL.mult)
                atts[(g, hp)] = att

            def unit_y(g, att, t):
                g0 = g * GS
                pso = [yps.tile([P, GS], FP32, tag="yp", name="yp")
                       for o in range(2)]
                for s in range(4):
                    for o in range(2):
                        nc.tensor.matmul(
                            pso[o], lhsT=(att[s][:, t * P:(t + 1) * P]),
                            rhs=(wo_sb[s][:, o * GS:(o + 1) * GS]),
                            start=(s == 0), stop=(s == 3))
                for o in range(2):
                    y_sb = ypool.tile([P, GS], BF16, tag=f"ysb{o}",
                                      name=f"ysb{o}")
                    nc.scalar.copy(y_sb, pso[o])
                    nc.scalar.dma_start(
                        out=y[g0 + t * P: g0 + (t + 1) * P,
                              o * GS:(o + 1) * GS],
                        in_=y_sb)

            # group order: spilled (light) groups alternate with inline
            # (heavy) ones; prepare(GORD[j+2]) = DMA readback for light,
            # qproj+elu for heavy
            GORD = [0, 1, 4, 2, 5, 3, 6, 7][:NG]

            def prepare(g):
                if g < SPILL_G:
                    if (g, 0, 0) not in qPs:
                        readback_qP(g)
                    return []
                return [(h, s) for h in range(HPC) for s in range(2)]

            for j in range(-2, 0):    # prologue: groups at positions 0/1
                if j + 2 < len(GORD):
                    qp_units = prepare(GORD[j + 2])
                    for iq, (h, s) in enumerate(qp_units):
                        unit_qproj(GORD[j + 2], h, s, iq, qpps)

            for j in range(NG + 1):
                ga = GORD[j] if j < NG else None
                gy = GORD[j - 1] if 1 <= j <= NG else None
                qp_units = prepare(GORD[j + 2]) if j + 2 < NG else []
                att_y = [atts.pop((gy, hp)) for hp in range(4)] \
                    if gy is not None else None
                iq = 0
                for i in range(4):
                    if ga is not None:
                        unit_attn(ga, i)
                    for _ in range(4):
                        if qp_units:
                            h, s = qp_units.pop(0)
                            unit_qproj(GORD[j + 2], h, s, iq, qpps)
                            iq += 1
                    if att_y is not None:
                        unit_y(gy, att_y, i)


def build(n=SEQ):
    # Bacc (not raw Bass): its compile pipeline splits multi-waits into
    # event semaphores (TRN2 allows at most 1 sync wait per instruction).
    nc = bacc.Bacc("TRN2", target_bir_lowering=False, debug=False,
                   enable_asserts=False)
    xT = nc.declare_dram_parameter("xT", [D, n], BF16, isOutput=False)
    wq = nc.declare_dram_parameter("wq", [D, DH], BF16, isOutput=False)
    wk = nc.declare_dram_parameter("wk", [D, DH], BF16, isOutput=False)
    wv = nc.declare_dram_parameter("wv", [D, DH], BF16, isOutput=False)
    proj = nc.declare_dram_parameter("proj", [DH, F], BF16, isOutput=False)
    wout = nc.declare_dram_parameter("wout", [DH, D], BF16, isOutput=False)
    y = nc.declare_dram_parameter("y", [n, D], BF16, isOutput=True)
    qPd = nc.dram_tensor("qPd", [2 * DH * 2, n], BF16)
    with tile.TileContext(nc) as tc:
        _emit(tc, n, xT, wq, wk, wv, proj, wout, y, qPd)
    nc.finalize()
    return nc


def make_in_maps(x, w_qkv, proj_matrix, w_out):
    bf = np.float16
    x = np.asarray(x, np.float32)
    w_qkv = np.asarray(w_qkv, bf)
    proj_matrix = np.asarray(proj_matrix, bf)
    w_out = np.asarray(w_out, bf)
    in_maps = []
    for c in range(NCORES):
        b, g = c // 2, c % 2
        in_maps.append({
            "xT": np.ascontiguousarray(x[b].T.astype(bf)),
            "wq": np.ascontiguousarray(w_qkv[:, DH * g:DH * (g + 1)]),
            "wk": np.ascontiguousarray(w_qkv[:, D + DH * g:D + DH * (g + 1)]),
            "wv": np.ascontiguousarray(w_qkv[:, 2 * D + DH * g:2 * D + DH * (g + 1)]),
            "proj": np.ascontiguousarray(
                proj_matrix[HPC * g:HPC * (g + 1)].reshape(DH, F)),
            "wout": np.ascontiguousarray(w_out[DH * g:DH * (g + 1), :]),
        })
    return in_maps


_NC_CACHE = {}


def get_nc(n=SEQ):
    if n not in _NC_CACHE:
        _NC_CACHE[n] = build(n)
    return _NC_CACHE[n]


def _install_ntff_hook_shim():
    """The agent image's antenv lacks axon_hooks; recreate it so
    run_bass_kernel_spmd(trace=True) can capture NTFF profiles."""
    import sys
    import types
    try:
        from antenv.axon_hooks import get_axon_ntff_profile_hook  # noqa: F401
        return True
    except ImportError:
        pass
    try:
        from trn_agent_boot.trn_boot import _ntff_profile_via_ctypes
        import antenv
        mod = types.ModuleType("antenv.axon_hooks")
        mod._hook = _ntff_profile_via_ctypes("/opt/axon/libaxon_pjrt.so")
        mod.set_axon_ntff_profile_hook = lambda h: setattr(mod, "_hook", h)
        mod.get_axon_ntff_profile_hook = lambda: mod._hook
        sys.modules["antenv.axon_hooks"] = mod
        antenv.axon_hooks = mod
        return True
    except Exception as e:  # profiling is best-effort
        print(f"ntff hook shim failed: {e}")
        return False


def run(x, w_qkv, proj_matrix, w_out, b_out, trace=False, **kw):
    if trace:
        _install_ntff_hook_shim()
    nc = get_nc(SEQ)
    in_maps = make_in_maps(x, w_qkv, proj_matrix, w_out)
    res = run_bass_kernel_spmd(nc, in_maps, list(range(NCORES)),
                               trace=trace, **kw)
    b_out = np.asarray(b_out, np.float32)
    out = np.empty((B, SEQ, D), np.float32)
    for b in range(B):
        out[b] = np.asarray(res.results[2 * b]["y"], np.float32) \
            + np.asarray(res.results[2 * b + 1]["y"], np.float32) \
            + b_out[None, :]
    return out, res


def kernel(x, w_qkv, proj_matrix, w_out, b_out):
    out, _ = run(x, w_qkv, proj_matrix, w_out, b_out)
    return out
